# revision 1
# baseline (speedup 1.0000x reference)
"""Trainium2 Bass kernel for AdvancedTransformerEncoderBlock.

Sharding: token-parallel across 8 cores (B=2 x 4 seq chunks of 512), each core
recomputes a 256-token K/V halo -> zero collectives. Per-core work:
  LN1 -> QKV (q/k transposed layout, v natural) -> RoPE -> local causal
  attention (window 256) -> out-proj + residual -> LN2 -> SwiGLU MLP + residual.
LN scale/shift params are folded into the weights/biases on the host.

All matmul operands are bf16 (halves weight DMA, full-rate PE at any tile
size); PSUM accumulation stays fp32, residual stream stays fp32.
RoPE rotate-half runs as a PE permutation matmul; the attention band mask and
the v/down biases are folded into PSUM via identity / ones-row matmuls, so
softmax needs no DVE mask add and exp reads PSUM directly.
"""

import numpy as np

B, S, D, F, H, HD = 2, 2048, 1024, 4096, 16, 64
WIN = 256
NCORES = 8
CH = 4           # chunks per batch
CS = S // CH     # 512 tokens per chunk (queries)
HT = CS + WIN    # 768 tokens incl. halo (keys/values)
NQT = CS // 128  # 4 query tiles
NKT = HT // 128  # 6 key tiles
EPS = 1e-5
NEG = -1e9


def build_program():
    import concourse.bass as bass
    import concourse.bacc as bacc_mod
    import concourse.tile as tile
    import concourse.mybir as mybir
    from concourse.masks import make_identity
    from contextlib import ExitStack

    dt = mybir.dt
    f32, bf16 = dt.float32, dt.bfloat16
    AF = mybir.ActivationFunctionType
    OP = mybir.AluOpType

    nc = bacc_mod.Bacc()
    P = lambda name, shape: nc.declare_dram_parameter(name, list(shape), f32, isOutput=False)
    Pb = lambda name, shape: nc.declare_dram_parameter(name, list(shape), bf16, isOutput=False)

    xh_d = Pb("xh", (HT, D))
    wq_d = Pb("wq", (8, 128, 8, 128))      # [mt][p=k][kt][m]
    wk_d = Pb("wk", (8, 128, 8, 128))
    wv_d = Pb("wv", (2, 8, 128, 512))      # [ch][kt][p][n] v-weight halves
    wo_d = Pb("wo", (8, 128, D))
    wg_d = Pb("wg", (32, 128, 8, 128))
    wu_d = Pb("wu", (32, 128, 8, 128))
    wd_d = Pb("wd", (32, 128, D))
    bv_d = Pb("bv", (1, D))
    bd_d = Pb("bd", (1, D))
    # packed constants: one f32 blob (biases, host pre-transposed) and one
    # bf16 blob (rope tables, masks, rotate-half permutation) -> 2 DMAs
    cbf_d = P("cbf", (128, 80))
    cbb_d = Pb("cbb", (128, 4224))
    out_d = nc.declare_dram_parameter("out", [CS, D], f32, isOutput=True)

    with tile.TileContext(nc) as tc, ExitStack() as top:
        const = top.enter_context(tc.tile_pool(name="const", bufs=1))

        # x tiles first: their DMAs head the queue so LN/transposes start early
        x_pool = top.enter_context(tc.tile_pool(name="x", bufs=6))
        x_tiles = []
        for tt in range(6):
            xt = x_pool.tile([128, D], bf16, tag="xt")
            # split dispatch SP/Pool: Pool's SWDGE path bypasses the HWDGE
            eng = nc.sync if tt % 2 == 0 else nc.gpsimd
            if tt == 0:
                # halves so bn_stats on cols 0-511 starts as soon as possible
                eng.dma_start(out=xt[:, 0:512], in_=xh_d[0:128, 0:512])
                eng.dma_start(out=xt[:, 512:1024], in_=xh_d[0:128, 512:1024])
            else:
                eng.dma_start(out=xt, in_=xh_d[tt * 128:(tt + 1) * 128, :])
            x_tiles.append(xt)

        # ---- constants (two blob DMAs + AP slices) ----
        cbf = const.tile([128, 80], f32, tag="cbf")
        nc.sync.dma_start(out=cbf, in_=cbf_d[:, :])
        cbb = const.tile([128, 4224], bf16, tag="cbb")
        nc.sync.dma_start(out=cbb, in_=cbb_d[:, :])
        bqk_sb = cbf[:, 0:16]
        bg_sb = cbf[:, 16:48]
        bu_sb = cbf[:, 48:80]
        cosq = cbb[:, 0:512]
        msinq = cbb[:, 512:1024]
        cosk = cbb[:, 1024:1792]
        msink = cbb[:, 1792:2560]
        masks = [cbb[:, 2560 + qt * 384:2560 + (qt + 1) * 384] for qt in range(NQT)]
        pshuf = cbb[:, 4096:4224]

        identb = const.tile([128, 128], bf16, tag="identb")
        make_identity(nc, identb)
        ones_row = const.tile([1, 128], bf16, tag="ones_row")
        nc.vector.memset(ones_row, 1.0)
        eps_t = const.tile([128, 1], f32, tag="eps")
        nc.vector.memset(eps_t, EPS)
        bv_sb = const.tile([1, D], bf16, tag="bv")
        nc.sync.dma_start(out=bv_sb, in_=bv_d[:, :])
        bd_sb = const.tile([1, D], bf16, tag="bd")
        nc.sync.dma_start(out=bd_sb, in_=bd_d[:, :])

        # ---- persistent activation pools (LIFO: outermost live longest) ----
        x2_pool = top.enter_context(tc.tile_pool(name="x2", bufs=4))
        y2T_pool = top.enter_context(tc.tile_pool(name="y2T", bufs=8))
        o2_pool = top.enter_context(tc.tile_pool(name="o2", bufs=8))

        def ln_stats(src, tmp_pool):
            """bn stats for one 128-token tile -> mv [128, (mean, var)]."""
            stats = tmp_pool.tile([128, 2, 6], f32, tag="lnstats")
            mv = tmp_pool.tile([128, 2], f32, tag="lnmv")
            for sg in range(2):
                nc.vector.bn_stats(out=stats[:, sg, :], in_=src[:, sg * 512:(sg + 1) * 512])
            nc.vector.bn_aggr(out=mv, in_=stats)
            return mv

        def ln_norm(src, dst, mv, tmp_pool):
            """dst = (src - mean)*rsqrt(var+eps); the affine normalize runs
            on Act via per-partition scale/bias APs."""
            rs = tmp_pool.tile([128, 1], f32, tag="lnrs")
            nc.scalar.activation(out=rs, in_=mv[:, 1:2], func=AF.Sqrt,
                                 bias=eps_t, scale=1.0)
            nc.vector.reciprocal(out=rs, in_=rs)
            nb = tmp_pool.tile([128, 1], f32, tag="lnnb")
            nc.vector.tensor_scalar(out=nb, in0=mv[:, 0:1], scalar1=rs,
                                    scalar2=-1.0, op0=OP.mult, op1=OP.mult)
            nc.scalar.activation(out=dst, in_=src, func=AF.Identity,
                                 bias=nb, scale=rs)

        def layernorm(src, dst, tmp_pool):
            ln_norm(src, dst, ln_stats(src, tmp_pool), tmp_pool)

        yT = []
        qT, kT, v_bf = [], [], []

        qkv_scope = ExitStack()
        yT_pool = qkv_scope.enter_context(tc.tile_pool(name="yT", bufs=8))
        qT_pool = qkv_scope.enter_context(tc.tile_pool(name="qT", bufs=8))
        kT_pool = qkv_scope.enter_context(tc.tile_pool(name="kT", bufs=8))
        vb_pool = qkv_scope.enter_context(tc.tile_pool(name="vb", bufs=6))

        # =========== phase 1a: LN1, y^T (bf16) ===========
        with ExitStack() as ph:
            ln_tmp = ph.enter_context(tc.tile_pool(name="ln_tmp", bufs=6))
            y_pool = ph.enter_context(tc.tile_pool(name="y", bufs=6))
            pst = ph.enter_context(tc.tile_pool(name="pst", bufs=6, space="PSUM"))

            for dtile in range(8):
                yT.append(yT_pool.tile([128, HT], bf16, name="yT", tag="yT"))
            ys = []
            for tt in range(6):
                y = y_pool.tile([128, D], bf16, tag="y")
                layernorm(x_tiles[tt], y, ln_tmp)
                ys.append(y)
            # dtile-outer transposes -> one wide copy per yT tile
            for dtile in range(8):
                pt = pst.tile([128, 6, 128], bf16, tag="pst")
                for tt in range(6):
                    nc.tensor.transpose(pt[:, tt, :],
                                        ys[tt][:, dtile * 128:(dtile + 1) * 128], identb)
                if dtile % 2 == 0:
                    nc.scalar.copy(out=yT[dtile], in_=pt)
                else:
                    nc.vector.tensor_copy(out=yT[dtile], in_=pt)

        # =========== phase 1b: v projection (natural layout, bf16) ===========
        with ExitStack() as ph:
            wv_pool = ph.enter_context(tc.tile_pool(name="wv", bufs=4))
            psv = ph.enter_context(tc.tile_pool(name="psv", bufs=6, space="PSUM"))

            for tt in range(6):
                v_bf.append(vb_pool.tile([128, D], bf16, name="vbf", tag="vbf"))
            for chv in range(2):
                sl = slice(chv * 512, (chv + 1) * 512)
                pv = [psv.tile([128, 512], f32, name="psv", tag="psv") for _ in range(6)]
                for kt in range(8):
                    w = wv_pool.tile([128, 512], bf16, tag="wv")
                    eng = nc.gpsimd if chv == 0 else nc.sync
                    eng.dma_start(out=w, in_=wv_d[chv, kt])
                    for tt in range(6):
                        nc.tensor.matmul(pv[tt], lhsT=yT[kt][:, tt * 128:(tt + 1) * 128],
                                         rhs=w, start=(kt == 0), stop=False)
                for tt in range(6):
                    # += bias via ones-row matmul, closes the accumulation
                    nc.tensor.matmul(pv[tt], lhsT=ones_row, rhs=bv_sb[:, sl],
                                     start=False, stop=True)
                for tt in range(6):
                    if tt % 2 == 0:
                        nc.scalar.copy(out=v_bf[tt][:, sl], in_=pv[tt])
                    else:
                        nc.vector.tensor_copy(out=v_bf[tt][:, sl], in_=pv[tt])

        # ==== phase 2: q/k projections + RoPE software-pipelined with ====
        # ==== attention: proj(mt+1) matmuls fill attn(mt) chain stalls ====
        with ExitStack() as ph:
            wqk_pool = ph.enter_context(tc.tile_pool(name="wqk", bufs=6))
            psb = ph.enter_context(tc.tile_pool(name="psb", bufs=2, space="PSUM"))
            rope_tmp = ph.enter_context(tc.tile_pool(name="rope_tmp", bufs=3))
            at = ph.enter_context(tc.tile_pool(name="at", bufs=8))
            attn_ph = ExitStack()
            psl = attn_ph.enter_context(tc.tile_pool(name="psl", bufs=3, space="PSUM"))
            pstr = attn_ph.enter_context(tc.tile_pool(name="pstr", bufs=2, space="PSUM"))
            pso = attn_ph.enter_context(tc.tile_pool(name="pso", bufs=1, space="PSUM"))

            o2 = [o2_pool.tile([128, CS], bf16, name="o2", tag="o2") for _ in range(8)]

            def rope_pe(dst_slice, src_slice, pr, w):
                nc.tensor.matmul(pr[:, :w], lhsT=pshuf, rhs=src_slice,
                                 start=True, stop=True)

            def proj_chunks(mt):
                """Projection+RoPE for q/k tile mt as a list of emit-closures;
                interleaved between attention stages of tile mt-1."""
                w_q = wqk_pool.tile([128, 8, 128], bf16, tag="wqk")
                nc.sync.dma_start(out=w_q, in_=wq_d[mt])
                w_k = wqk_pool.tile([128, 8, 128], bf16, tag="wqk")
                nc.sync.dma_start(out=w_k, in_=wk_d[mt])
                qt_t = qT_pool.tile([128, CS], bf16, tag="qT")
                kt_t = kT_pool.tile([128, HT], bf16, tag="kT")
                st = {}

                def c0():  # q projection
                    ps = psb.tile([128, CS], f32, tag="psqk")
                    for kt in range(8):
                        nc.tensor.matmul(ps, lhsT=w_q[:, kt, :], rhs=yT[kt][:, WIN:HT],
                                         start=(kt == 0), stop=(kt == 7))
                    qb = rope_tmp.tile([128, HT], bf16, tag="ropesrc")
                    nc.scalar.activation(out=qb[:, :CS], in_=ps, func=AF.Identity,
                                         bias=bqk_sb[:, mt:mt + 1], scale=1.0)
                    st["qb"] = qb

                def c1():  # q rope
                    qb = st["qb"]
                    pr = psb.tile([128, 512], f32, tag="psqk")
                    rope_pe(None, qb[:, :CS], pr, CS)
                    u = rope_tmp.tile([128, HT], bf16, tag="ropeu")
                    nc.vector.tensor_mul(out=u[:, :CS], in0=qb[:, :CS], in1=cosq)
                    t1 = rope_tmp.tile([128, 512], bf16, tag="ropet")
                    nc.vector.tensor_mul(out=t1, in0=pr, in1=msinq)
                    nc.vector.tensor_add(out=qt_t, in0=u[:, :CS], in1=t1)

                def c2():  # k projection half 0
                    kb = rope_tmp.tile([128, HT], bf16, tag="ropesrc")
                    st["kb"] = kb
                    ps = psb.tile([128, 384], f32, tag="psqk")
                    for kt in range(8):
                        nc.tensor.matmul(ps, lhsT=w_k[:, kt, :], rhs=yT[kt][:, 0:384],
                                         start=(kt == 0), stop=(kt == 7))
                    nc.scalar.activation(out=kb[:, 0:384], in_=ps, func=AF.Identity,
                                         bias=bqk_sb[:, 8 + mt:9 + mt], scale=1.0)

                def c3():  # k projection half 1 + k rope
                    kb = st["kb"]
                    ps = psb.tile([128, 384], f32, tag="psqk")
                    for kt in range(8):
                        nc.tensor.matmul(ps, lhsT=w_k[:, kt, :], rhs=yT[kt][:, 384:768],
                                         start=(kt == 0), stop=(kt == 7))
                    nc.scalar.activation(out=kb[:, 384:768], in_=ps, func=AF.Identity,
                                         bias=bqk_sb[:, 8 + mt:9 + mt], scale=1.0)
                    u = rope_tmp.tile([128, HT], bf16, tag="ropeu")
                    nc.vector.tensor_mul(out=u, in0=kb, in1=cosk)
                    for c in range(2):
                        w = 512 if c == 0 else 256
                        sl_ = slice(c * 512, c * 512 + w)
                        pr = psb.tile([128, 512], f32, tag="psqk")
                        rope_pe(None, kb[:, sl_], pr, w)
                        t1 = rope_tmp.tile([128, 512], bf16, tag="ropet")
                        nc.vector.tensor_mul(out=t1[:, :w], in0=pr[:, :w],
                                             in1=msink[:, sl_])
                        nc.vector.tensor_add(out=kt_t[:, sl_], in0=u[:, sl_],
                                             in1=t1[:, :w])

                qT.append(qt_t)
                kT.append(kt_t)
                return [c0, c1, c2, c3]

            def attn_step(mt, qt, filler):
                """One query tile of attention for head-pair mt, with PE filler
                closures injected between dependent stages."""
                ps_l2, E2, sums2 = [], [], []
                for hh in range(2):
                    hr = hh * 64
                    ps_l = psl.tile([128, 384], f32, tag="psl")
                    nc.tensor.matmul(ps_l, lhsT=identb, rhs=masks[qt],
                                     start=True, stop=False)
                    nc.tensor.matmul(ps_l,
                                     lhsT=qT[mt][hr:hr + 64, qt * 128:(qt + 1) * 128],
                                     rhs=kT[mt][hr:hr + 64, qt * 128:qt * 128 + 384],
                                     start=False, stop=True)
                    ps_l2.append(ps_l)
                for hh in range(2):
                    E = at.tile([128, 384], bf16, tag="E")
                    sums = at.tile([128, 1], f32, tag="sums")
                    nc.scalar.activation(out=E, in_=ps_l2[hh], func=AF.Exp,
                                         scale=float(HD) ** -0.5, accum_out=sums)
                    E2.append(E)
                    sums2.append(sums)
                if filler:
                    filler[0]()          # PE filler while exp runs
                ps_t = pstr.tile([128, 2, 384], bf16, tag="pstr")
                for hh in range(2):
                    sums = sums2[hh]
                    nc.vector.reciprocal(out=sums, in_=sums)
                    En = at.tile([128, 384], bf16, tag="En")
                    nc.vector.tensor_scalar_mul(out=En, in0=E2[hh], scalar1=sums)
                    for j in range(3):
                        nc.tensor.transpose(ps_t[:, hh, j * 128:(j + 1) * 128],
                                            En[:, j * 128:(j + 1) * 128], identb)
                ET = at.tile([128, 2, 384], bf16, tag="ET")
                if (mt + qt) % 2 == 0:
                    nc.vector.tensor_copy(out=ET, in_=ps_t)
                else:
                    nc.scalar.copy(out=ET, in_=ps_t)
                if len(filler) > 1:
                    filler[1]()          # PE filler while ET copies drain
                ps_o = pso.tile([128, 128], f32, tag="pso")
                for hh in range(2):
                    h = 2 * mt + hh
                    hr = hh * 64
                    for j in range(3):
                        nc.tensor.matmul(ps_o[hr:hr + 64, :],
                                         lhsT=v_bf[qt + j][:, h * 64:h * 64 + 64],
                                         rhs=ET[:, hh, j * 128:(j + 1) * 128],
                                         start=(j == 0), stop=(j == 2))
                if (mt + qt) % 2 == 0:
                    nc.scalar.copy(out=o2[mt][:, qt * 128:(qt + 1) * 128], in_=ps_o)
                else:
                    nc.vector.tensor_copy(out=o2[mt][:, qt * 128:(qt + 1) * 128],
                                          in_=ps_o)

            # ---- phase 3 resources (shared with phase 2 for interleaving) ----
            wo_pool = ph.enter_context(tc.tile_pool(name="wo", bufs=8))
            ln_tmp2 = ph.enter_context(tc.tile_pool(name="ln_tmp2", bufs=3))
            y2_pool = ph.enter_context(tc.tile_pool(name="y2", bufs=4))
            wo_sb = []
            x2_list = [None] * NQT
            mv2_list = [None] * NQT

            def load_wo():
                for dtile in range(8):
                    w = wo_pool.tile([128, D], bf16, tag="wo")
                    nc.gpsimd.dma_start(out=w, in_=wo_d[dtile])
                    wo_sb.append(w)

            def outproj_chunk(qt):
                def f():
                    x2 = x2_pool.tile([128, D], f32, tag="x2")
                    for ch2 in range(2):
                        sl = slice(ch2 * 512, (ch2 + 1) * 512)
                        ps = psb.tile([128, 512], f32, tag="psqk")
                        for dtile in range(8):
                            nc.tensor.matmul(ps,
                                             lhsT=o2[dtile][:, qt * 128:(qt + 1) * 128],
                                             rhs=wo_sb[dtile][:, sl],
                                             start=(dtile == 0), stop=(dtile == 7))
                        nc.vector.tensor_add(out=x2[:, sl], in0=ps,
                                             in1=x_tiles[2 + qt][:, sl])
                    x2_list[qt] = x2
                    mv2_list[qt] = ln_stats(x2, ln_tmp2)
                return f

            y2_list = [None] * NQT

            def lnfin_chunk(qt):
                def f():
                    y2 = y2_pool.tile([128, D], bf16, tag="y2")
                    ln_norm(x2_list[qt], y2, mv2_list[qt], ln_tmp2)
                    y2_list[qt] = y2
                return f

            chunks = proj_chunks(0)
            for c in chunks:
                c()
            fill_plan = {
                (7, 1): [outproj_chunk(0)],
                (7, 2): [outproj_chunk(1)],
                (7, 3): [outproj_chunk(2)],
            }
            for mt in range(8):
                if mt + 1 < 8:
                    nxt = proj_chunks(mt + 1)
                    if mt + 1 == 2:
                        load_wo()
                for qt in range(NQT):
                    if mt + 1 < 8:
                        filler = [nxt[qt]]
                    else:
                        filler = fill_plan.get((mt, qt), [])
                    attn_step(mt, qt, filler)
            outproj_chunk(NQT - 1)()
            lnfin_chunk(0)()
            lnfin_chunk(1)()
            lnfin_chunk(2)()
            attn_ph.close()

            # ---- y2^T transposes (dtile-outer: one wide copy per dtile) ----
            pst2 = ph.enter_context(tc.tile_pool(name="pst2", bufs=6, space="PSUM"))
            y2T = [y2T_pool.tile([128, CS], bf16, name="y2T", tag="y2T") for _ in range(8)]

            def y2t_transposes(dtiles):
                for dtile in dtiles:
                    pt = pst2.tile([128, 4, 128], bf16, tag="pst2b")
                    for qt in range(NQT):
                        nc.tensor.transpose(pt[:, qt, :],
                                            y2_list[qt][:, dtile * 128:(dtile + 1) * 128],
                                            identb)
                    if dtile % 2 == 0:
                        nc.scalar.copy(out=y2T[dtile], in_=pt)
                    else:
                        nc.vector.tensor_copy(out=y2T[dtile], in_=pt)

            lnfin_chunk(3)()
            y2t_transposes(range(8))

        qkv_scope.close()

        # =========== phase 4: MLP gate/up -> H ===========
        Hs = []
        mlp_scope = ExitStack()
        hh_pool = mlp_scope.enter_context(tc.tile_pool(name="hh", bufs=32))
        with ExitStack() as ph:
            wgu_pool = ph.enter_context(tc.tile_pool(name="wgu", bufs=6))
            psg = ph.enter_context(tc.tile_pool(name="psg", bufs=4, space="PSUM"))
            gu_tmp = ph.enter_context(tc.tile_pool(name="gu_tmp", bufs=6))

            for mt in range(32):
                wg_sb = wgu_pool.tile([128, 8, 128], bf16, tag="wgu")
                nc.sync.dma_start(out=wg_sb, in_=wg_d[mt])
                wu_sb = wgu_pool.tile([128, 8, 128], bf16, tag="wgu")
                nc.sync.dma_start(out=wu_sb, in_=wu_d[mt])
                ps_g = psg.tile([128, CS], f32, tag="psgu")
                ps_u = psg.tile([128, CS], f32, tag="psgu")
                for kt in range(8):
                    nc.tensor.matmul(ps_g, lhsT=wg_sb[:, kt, :], rhs=y2T[kt],
                                     start=(kt == 0), stop=(kt == 7))
                for kt in range(8):
                    nc.tensor.matmul(ps_u, lhsT=wu_sb[:, kt, :], rhs=y2T[kt],
                                     start=(kt == 0), stop=(kt == 7))
                G = gu_tmp.tile([128, CS], bf16, tag="G")
                nc.scalar.activation(out=G, in_=ps_g, func=AF.Identity,
                                     bias=bg_sb[:, mt:mt + 1], scale=1.0)
                U = gu_tmp.tile([128, CS], bf16, tag="U")
                nc.scalar.activation(out=U, in_=ps_u, func=AF.Silu,
                                     bias=bu_sb[:, mt:mt + 1], scale=1.0)
                Ht = hh_pool.tile([128, CS], bf16, tag="hh")
                nc.vector.tensor_mul(out=Ht, in0=G, in1=U)
                Hs.append(Ht)

        # =========== phase 5: down proj + residual + store ===========
        with ExitStack() as ph:
            wd_pool = ph.enter_context(tc.tile_pool(name="wd", bufs=5))
            psd = ph.enter_context(tc.tile_pool(name="psd", bufs=8, space="PSUM"))
            out_pool = ph.enter_context(tc.tile_pool(name="outp", bufs=4))

            ps_d = [psd.tile([128, 512], f32, name="psd", tag="psd") for _ in range(8)]
            for kt in range(31):
                w = wd_pool.tile([128, D], bf16, tag="wd")
                nc.sync.dma_start(out=w, in_=wd_d[kt])
                for tt in range(NQT):
                    for ch3 in range(2):
                        nc.tensor.matmul(ps_d[tt * 2 + ch3],
                                         lhsT=Hs[kt][:, tt * 128:(tt + 1) * 128],
                                         rhs=w[:, ch3 * 512:(ch3 + 1) * 512],
                                         start=(kt == 0), stop=False)
            # final k-tile: close/ship each token tile as soon as it finishes
            w = wd_pool.tile([128, D], bf16, tag="wd")
            nc.sync.dma_start(out=w, in_=wd_d[31])
            for tt in range(NQT):
                ot = out_pool.tile([128, D], f32, tag="outp")
                for ch3 in range(2):
                    sl = slice(ch3 * 512, (ch3 + 1) * 512)
                    nc.tensor.matmul(ps_d[tt * 2 + ch3],
                                     lhsT=Hs[31][:, tt * 128:(tt + 1) * 128],
                                     rhs=w[:, sl], start=False, stop=False)
                    nc.tensor.matmul(ps_d[tt * 2 + ch3], lhsT=ones_row,
                                     rhs=bd_sb[:, sl], start=False, stop=True)
                    nc.vector.tensor_add(out=ot[:, sl], in0=ps_d[tt * 2 + ch3],
                                         in1=x2_list[tt][:, sl])
                nc.sync.dma_start(out=out_d[tt * 128:(tt + 1) * 128, :], in_=ot)
        mlp_scope.close()

    nc.compile()
    return nc


def prep_inputs(x, w_qkv, w_out, g1, b1, g2, b2, w_gate, b_gate, w_up, b_up,
                w_down, b_down):
    """Host-side: fold LN params into weights, pre-tile, build per-core tensors."""
    import ml_dtypes
    f32 = np.float32
    bf16 = ml_dtypes.bfloat16

    def tile_lhsT(w):  # [D, M] -> [mt, p, kt, m]
        Dd, M = w.shape
        return np.ascontiguousarray(
            w.reshape(Dd // 128, 128, M // 128, 128).transpose(2, 1, 0, 3)).astype(bf16)

    wqkv_f = (w_qkv * g1[:, None]).astype(f32)
    bqkv = (b1 @ w_qkv).astype(f32)
    common = {
        "wq": tile_lhsT(wqkv_f[:, :1024]),
        "wk": tile_lhsT(wqkv_f[:, 1024:2048]),
        "wv": np.ascontiguousarray(
            wqkv_f[:, 2048:3072].reshape(8, 128, 2, 512).transpose(2, 0, 1, 3)).astype(bf16),
        "wo": np.ascontiguousarray(w_out.reshape(8, 128, D)).astype(bf16),
        "wg": tile_lhsT((w_gate * g2[:, None]).astype(f32)),
        "wu": tile_lhsT((w_up * g2[:, None]).astype(f32)),
        "wd": np.ascontiguousarray(w_down.reshape(32, 128, D)).astype(bf16),
        "bv": bqkv[2048:].reshape(1, D).astype(bf16),
        "bd": b_down.reshape(1, D).astype(bf16),
    }
    bqk_pt = bqkv[:2048].reshape(16, 128).T          # [p, t]
    bg_pt = (b_gate + b2 @ w_gate).astype(f32).reshape(32, 128).T
    bu_pt = (b_up + b2 @ w_up).astype(f32).reshape(32, 128).T
    common["cbf"] = np.ascontiguousarray(
        np.concatenate([bqk_pt, bg_pt, bu_pt], axis=1)).astype(f32)

    # rotate-half permutation as lhsT: rot[m] = sign(m) * src[sigma(m)]
    # sign folded into the sin tables instead -> pshuf is a pure permutation.
    pshuf = np.zeros((128, 128), f32)
    for m in range(128):
        base = (m // 64) * 64
        r = m % 64
        sig = base + (r + 32) % 64
        pshuf[sig, m] = 1.0
    pshuf = pshuf.astype(bf16)

    half = HD // 2
    inv_freq = 1.0 / (10000.0 ** (np.arange(half, dtype=np.float64) / half))

    def rope_tables(pos):
        t = np.maximum(pos, 0).astype(np.float64)
        freqs = np.outer(t, inv_freq)            # [T, 32]
        emb = np.concatenate([freqs, freqs], 1)  # [T, 64]
        c = np.cos(emb).T.astype(f32)            # [64, T]
        s = np.sin(emb).T.astype(f32)
        # sign-folded sin: rows 0-31 get -sin (they receive -x2), rows 32-63 +sin
        ms = s.copy()
        ms[:32] = -ms[:32]
        return (np.ascontiguousarray(np.vstack([c, c])),
                np.ascontiguousarray(np.vstack([ms, ms])))

    in_maps = []
    for c in range(NCORES):
        b, chunk = c // CH, c % CH
        q0 = chunk * CS
        lo = q0 - WIN
        xh = np.zeros((HT, D), f32)
        src_lo = max(0, lo)
        xh[src_lo - lo:] = x[b, src_lo:q0 + CS]
        xh = xh.astype(bf16)
        pos_k = np.arange(lo, q0 + CS)
        cosk_a, sink_a = rope_tables(pos_k)
        cosq_a = np.ascontiguousarray(cosk_a[:, WIN:]).astype(bf16)
        sinq_a = np.ascontiguousarray(sink_a[:, WIN:]).astype(bf16)
        # mask [qt, r, c]: query i = q0 + qt*128 + r ; key j = lo + qt*128 + cc
        qt_i = np.arange(NQT)[:, None, None]
        r_i = np.arange(128)[None, :, None]
        c_i = np.arange(384)[None, None, :]
        gi = q0 + qt_i * 128 + r_i
        gj = lo + qt_i * 128 + c_i
        valid = (gj <= gi) & (gi - gj <= WIN) & (gj >= 0)
        mask = np.where(valid, 0.0, NEG).astype(bf16)  # [4, 128, 384]
        cbb = np.concatenate(
            [cosq_a, sinq_a, cosk_a.astype(bf16), sink_a.astype(bf16),
             mask.transpose(1, 0, 2).reshape(128, 4 * 384), pshuf], axis=1)
        in_maps.append(dict(common, xh=xh, cbb=np.ascontiguousarray(cbb)))
    return in_maps


_PROG = {}


def kernel(**inputs):
    from concourse.bass_utils import run_bass_kernel_spmd

    inputs = {k: np.asarray(v, dtype=np.float32) for k, v in inputs.items()}
    in_maps = prep_inputs(**inputs)
    if "nc" not in _PROG:
        _PROG["nc"] = build_program()
    nc = _PROG["nc"]
    res = run_bass_kernel_spmd(nc, in_maps, core_ids=list(range(NCORES)))
    out = np.zeros((B, S, D), np.float32)
    for c in range(NCORES):
        b, chunk = c // CH, c % CH
        out[b, chunk * CS:(chunk + 1) * CS] = res.results[c]["out"]
    return out



# revision 24
# speedup vs baseline: 1.2406x; 1.2406x over previous
"""Trainium2 Bass kernel for AdvancedTransformerEncoderBlock (fp8 DoubleRow).

Sharding: token-parallel across 8 cores (B=2 x 4 seq chunks of 512), each core
recomputes a 256-token K/V halo -> zero collectives.

Precision plan (validated vs fp32 reference, rel_err ~= 0.015):
  - qkv proj:   fp8e4 DoubleRow, weights split hi+lo(x16), activation split
                hi + hi/16 + residual  (3 passes, 4x per-pass speedup)
  - attention:  bf16 (transposed-logits flow: logits land [keys, queries] in
                PSUM; exp on Act; band mask folded into the PSUM->SBUF copy as
                a 0/1 multiply; softmax sums via ones[128,64] matmul so the
                per-query denominators arrive broadcast across partitions;
                normalize folded into the o2 copy)
  - out proj:   fp8e4 DoubleRow single-pass (o2/wo plain fp8)
  - gate/up:    like qkv (3 passes)
  - down proj:  weights split fp8(4w) + fp8(32*res), H plain fp8 + H/8 copy;
                the 4x weight prescale (keeps wd out of fp8 subnormals) is
                undone by a 0.25 scale folded into the PSUM->SBUF copy
PSUM accumulation stays fp32, residual stream stays fp32.
RoPE rotate-half runs as a PE permutation matmul.
Attention runs one query-tile ahead on logits so exp/mask latency hides under
sums/AV of the previous tile plus the interleaved projection fillers.
"""

import numpy as np

B, S, D, F, H, HD = 2, 2048, 1024, 4096, 16, 64
WIN = 256
NCORES = 8
CH = 4           # chunks per batch
CS = S // CH     # 512 tokens per chunk (queries)
HT = CS + WIN    # 768 tokens incl. halo (keys/values)
NQT = CS // 128  # 4 query tiles
EPS = 1e-5
QKV_THIRD = True   # include activation-residual pass in qkv proj
GU_THIRD = True    # include activation-residual pass in gate/up


def build_program(has_bv=False, has_bg=False, has_bd=False):
    import concourse.bass as bass
    import concourse.bacc as bacc_mod
    import concourse.tile as tile
    import concourse.mybir as mybir
    from concourse.masks import make_identity
    from contextlib import ExitStack

    dt = mybir.dt
    f32, bf16, f8 = dt.float32, dt.bfloat16, dt.float8e4
    AF = mybir.ActivationFunctionType
    OP = mybir.AluOpType
    DR = mybir.MatmulPerfMode.DoubleRow

    nc = bacc_mod.Bacc()
    Pf = lambda name, shape: nc.declare_dram_parameter(name, list(shape), f32, isOutput=False)
    Pb = lambda name, shape: nc.declare_dram_parameter(name, list(shape), bf16, isOutput=False)
    P8 = lambda name, shape: nc.declare_dram_parameter(name, list(shape), f8, isOutput=False)

    xh_d = Pb("xh", (HT, D))
    wqk_d = P8("wqk", (8, 128, 4, 4, 2, 128))   # [mt][p][qhi,qlo,khi,klo][pair][i][m]
    wv_d = P8("wv", (128, 2, 4, 2, 1024))       # [p][hi/lo][pair][i][n]
    wo_d = P8("wo", (128, 4, 2, 1024))          # [p][pair][i][n]
    wgu_d = P8("wgu", (32, 128, 2, 2, 4, 2, 128))  # [mt][p][g/u][hi/lo][pair][i][m]
    wd_d = P8("wd", (16, 128, 2, 2, 1024))      # [pair][p][hi/lo][i][n]
    bv_d = Pb("bv", (1, D))
    bd_d = Pb("bd", (1, D))
    bg_d = Pb("bg", (1, F))
    cbf_d = Pf("cbf", (128, 48))                # bqk [:,0:16], bu [:,16:48]
    cbb_d = Pb("cbb", (128, 4224))
    out_d = nc.declare_dram_parameter("out", [CS, D], f32, isOutput=True)

    with tile.TileContext(nc) as tc, ExitStack() as top:
        const = top.enter_context(tc.tile_pool(name="const", bufs=1))

        # x tiles first: their DMAs head the queue so LN/transposes start early
        x_pool = top.enter_context(tc.tile_pool(name="x", bufs=6))
        x_tiles = []
        for tt in range(6):
            xt = x_pool.tile([128, D], bf16, tag="xt")
            eng = nc.sync if tt % 2 == 0 else nc.gpsimd
            if tt == 0:
                eng.dma_start(out=xt[:, 0:512], in_=xh_d[0:128, 0:512])
                eng.dma_start(out=xt[:, 512:1024], in_=xh_d[0:128, 512:1024])
            else:
                eng.dma_start(out=xt, in_=xh_d[tt * 128:(tt + 1) * 128, :])
            x_tiles.append(xt)

        # ---- constants ----
        cbf = const.tile([128, 48], f32, tag="cbf")
        nc.sync.dma_start(out=cbf, in_=cbf_d[:, :])
        cbb = const.tile([128, 4224], bf16, tag="cbb")
        nc.gpsimd.dma_start(out=cbb, in_=cbb_d[:, :])
        bqk_sb = cbf[:, 0:16]
        bu_sb = cbf[:, 16:48]
        cosq = cbb[:, 0:512]
        msinq = cbb[:, 512:1024]
        cosk = cbb[:, 1024:1792]
        msink = cbb[:, 1792:2560]
        masks = [cbb[:, 2560 + qt * 384:2560 + (qt + 1) * 384] for qt in range(NQT)]
        pshuf = cbb[:, 4096:4224]

        identb = const.tile([128, 128], bf16, tag="identb")
        make_identity(nc, identb)
        ones64 = const.tile([128, 64], bf16, tag="ones64")
        nc.vector.memset(ones64, 1.0)
        ones_row = const.tile([1, 512], bf16, tag="ones_row")
        nc.vector.memset(ones_row, 1.0)
        eps_t = const.tile([128, 1], f32, tag="eps")
        nc.vector.memset(eps_t, EPS)
        if has_bv:
            bv_sb = const.tile([1, D], bf16, tag="bv")
            nc.sync.dma_start(out=bv_sb, in_=bv_d[:, :])
        if has_bd:
            bd_sb = const.tile([1, D], bf16, tag="bd")
            nc.sync.dma_start(out=bd_sb, in_=bd_d[:, :])
        if has_bg:
            bg_sb = const.tile([1, F], bf16, tag="bg")
            nc.sync.dma_start(out=bg_sb, in_=bg_d[:, :])

        # ---- persistent activation pools ----
        x2_pool = top.enter_context(tc.tile_pool(name="x2", bufs=4))
        y2T_pool = top.enter_context(tc.tile_pool(name="y2T", bufs=4))
        o2_pool = top.enter_context(tc.tile_pool(name="o2", bufs=4))

        def ln_stats(src, tmp_pool):
            stats = tmp_pool.tile([128, 2, 6], f32, tag="lnstats")
            mv = tmp_pool.tile([128, 2], f32, tag="lnmv")
            for sg in range(2):
                nc.vector.bn_stats(out=stats[:, sg, :], in_=src[:, sg * 512:(sg + 1) * 512])
            nc.vector.bn_aggr(out=mv, in_=stats)
            return mv

        def ln_norm(src, dst, mv, tmp_pool):
            rs = tmp_pool.tile([128, 1], f32, tag="lnrs")
            nc.scalar.activation(out=rs, in_=mv[:, 1:2], func=AF.Sqrt,
                                 bias=eps_t, scale=1.0)
            nc.vector.reciprocal(out=rs, in_=rs)
            nb = tmp_pool.tile([128, 1], f32, tag="lnnb")
            nc.vector.tensor_scalar(out=nb, in0=mv[:, 0:1], scalar1=rs,
                                    scalar2=-1.0, op0=OP.mult, op1=OP.mult)
            nc.scalar.activation(out=dst[:, 0:512], in_=src[:, 0:512],
                                 func=AF.Identity, bias=nb, scale=rs)
            nc.vector.tensor_scalar(out=dst[:, 512:1024], in0=src[:, 512:1024],
                                    scalar1=rs, scalar2=nb, op0=OP.mult,
                                    op1=OP.add)

        def layernorm(src, dst, tmp_pool):
            ln_norm(src, dst, ln_stats(src, tmp_pool), tmp_pool)

        # late-lived pools opened early for LIFO stack order (no instructions
        # are emitted at open time); wqk gets fresh persistent space so its
        # streaming DMAs never wait on freed-pool readers
        wqkp_scope = ExitStack()
        wqk_pool = wqkp_scope.enter_context(tc.tile_pool(name="wqkp", bufs=3))
        mlp_scope = ExitStack()
        hh_pool = mlp_scope.enter_context(tc.tile_pool(name="hh", bufs=16))
        wd_pool = mlp_scope.enter_context(tc.tile_pool(name="wd", bufs=4))
        wgu0_pool = mlp_scope.enter_context(tc.tile_pool(name="wgu0", bufs=2))

        # mid-lived: x2-LN workspace + y2 (read by the late y2T transposes)
        mid_scope = ExitStack()
        ln_tmp2 = mid_scope.enter_context(tc.tile_pool(name="ln_tmp2", bufs=3))
        y2_pool = mid_scope.enter_context(tc.tile_pool(name="y2", bufs=4))

        qkv_scope = ExitStack()
        yT_pool = qkv_scope.enter_context(tc.tile_pool(name="yT", bufs=4))
        wo_pool = qkv_scope.enter_context(tc.tile_pool(name="wo", bufs=1))
        wo8 = wo_pool.tile([128, 4, 2, 1024], f8, tag="wo8")
        qT_pool = qkv_scope.enter_context(tc.tile_pool(name="qT", bufs=3))
        kT_pool = qkv_scope.enter_context(tc.tile_pool(name="kT", bufs=3))
        vb_pool = qkv_scope.enter_context(tc.tile_pool(name="vb", bufs=6))

        # v-proj weights: DMAs queued ahead of everything but x (phase B is
        # the first weight consumer); wo follows on the same queue
        vw_scope = ExitStack()
        wv_pool = vw_scope.enter_context(tc.tile_pool(name="wv", bufs=1))
        wv8 = wv_pool.tile([128, 2, 4, 2, 1024], f8, tag="wv8")
        for p_ in range(4):
            eng = nc.sync if p_ % 2 == 0 else nc.gpsimd
            eng.dma_start(out=wv8[:, :, p_, :, :], in_=wv_d[:, :, p_, :, :])
        nc.gpsimd.dma_start(out=wo8, in_=wo_d[:, :, :, :])

        # y^T pair tiles split by token halves so consumers start after the
        # first three LN outputs: a = tokens 0:384, b = 384:768
        HH = HT // 2
        yT8a = [yT_pool.tile([128, 2, HH], f8, name="yT8a", tag="yT8a") for _ in range(4)]
        yT8b = [yT_pool.tile([128, 2, HH], f8, name="yT8b", tag="yT8b") for _ in range(4)]
        yT8sa = [yT_pool.tile([128, 2, HH], f8, name="yT8sa", tag="yT8sa") for _ in range(4)]
        yT8sb = [yT_pool.tile([128, 2, HH], f8, name="yT8sb", tag="yT8sb") for _ in range(4)]
        yTr8a = [yT_pool.tile([128, 2, HH], f8, name="yTr8a", tag="yTr8a")
                 for _ in range(4)] if QKV_THIRD else None
        yTr8b = [yT_pool.tile([128, 2, HH], f8, name="yTr8b", tag="yTr8b")
                 for _ in range(4)] if QKV_THIRD else None

        # =========== phase A: LN1 -> y -> y^T fp8 triplet ===========
        with ExitStack() as ph:
            ln_tmp = ph.enter_context(tc.tile_pool(name="ln_tmp", bufs=6))
            y_pool = ph.enter_context(tc.tile_pool(name="y", bufs=6))
            pst = ph.enter_context(tc.tile_pool(name="pst", bufs=8, space="PSUM"))

            ys = []
            for tt in range(6):
                y = y_pool.tile([128, D], bf16, tag="y")
                layernorm(x_tiles[tt], y, ln_tmp)
                ys.append(y)
            # tt-outer with half-granular copies: the a-half (tokens 0:384)
            # ships as soon as the first three LN outputs exist
            pts = [pst.tile([128, 6, 128], bf16, name="pt", tag="pst")
                   for _ in range(8)]
            for half, (hi_l, s_l, r_l) in enumerate(
                    [(yT8a, yT8sa, yTr8a), (yT8b, yT8sb, yTr8b)]):
                for tt in range(half * 3, half * 3 + 3):
                    for dtl in range(8):
                        nc.tensor.transpose(pts[dtl][:, tt, :],
                                            ys[tt][:, dtl * 128:(dtl + 1) * 128],
                                            identb)
                for dtl in range(8):
                    pt = pts[dtl][:, half * 3:half * 3 + 3, :]
                    dst_hi = hi_l[dtl // 2][:, dtl % 2, :]
                    nc.scalar.activation(out=dst_hi, in_=pt, func=AF.Identity)
                    nc.gpsimd.tensor_scalar_mul(out=s_l[dtl // 2][:, dtl % 2, :],
                                                in0=dst_hi, scalar1=1.0 / 16)
                    if QKV_THIRD:
                        nc.vector.tensor_tensor(out=r_l[dtl // 2][:, dtl % 2, :],
                                                in0=pt, in1=dst_hi, op=OP.subtract)

        qkv_terms = [(yT8a, yT8b, 0), (yT8sa, yT8sb, 1)] + \
            ([(yTr8a, yTr8b, 0)] if QKV_THIRD else [])

        def ytok(term, lo, hi):
            """AP for token range [lo, hi) of a qkv term (within one half)."""
            a_l, b_l, _ = term
            if hi <= HH:
                return lambda p: a_l[p][:, :, lo:hi]
            assert lo >= HH
            return lambda p: b_l[p][:, :, lo - HH:hi - HH]

        # =========== phase B: v projection (fp8 DR, pair-outer) ===========
        v_bf = []
        with ExitStack() as ph:
            psv = ph.enter_context(tc.tile_pool(name="psv", bufs=6, space="PSUM"))
            for tt in range(6):
                v_bf.append(vb_pool.tile([128, D], f8, name="vbf", tag="vbf"))
            nterm = len(qkv_terms)
            for chv in range(2):
                pv = [psv.tile([128, 512], f32, name="psv", tag="psv") for _ in range(6)]
                for ti, term in enumerate(qkv_terms):
                    hl = term[2]
                    for p in range(4):
                        for tt in range(6):
                            lsrc = ytok(term, tt * 128, (tt + 1) * 128)(p)
                            for cn in range(2):
                                reg = pv[tt][:, cn * 256:(cn + 1) * 256]
                                last = (p == 3 and ti == nterm - 1)
                                nc.tensor.matmul(
                                    reg,
                                    lhsT=lsrc,
                                    rhs=wv8[:, hl, p, :,
                                            chv * 512 + cn * 256:chv * 512 + (cn + 1) * 256],
                                    start=(ti == 0 and p == 0 and cn == 0),
                                    stop=(last and not has_bv), perf_mode=DR)
                if has_bv:
                    for tt in range(6):
                        for cn in range(2):
                            nc.tensor.matmul(pv[tt][:, cn * 256:(cn + 1) * 256],
                                             lhsT=ones_row[:, 0:128],
                                             rhs=bv_sb[:, chv * 512 + cn * 256:
                                                       chv * 512 + (cn + 1) * 256],
                                             start=False, stop=True)
                for tt in range(6):
                    sl = slice(chv * 512, (chv + 1) * 512)
                    if tt % 2 == 0:
                        nc.scalar.copy(out=v_bf[tt][:, sl], in_=pv[tt])
                    else:
                        nc.vector.tensor_copy(out=v_bf[tt][:, sl], in_=pv[tt])
        vw_scope.close()

        # ==== phase C: q/k proj + RoPE pipelined with attention (flow B) ====
        qT, kT = [], []
        with ExitStack() as ph:
            psb = ph.enter_context(tc.tile_pool(name="psb", bufs=3, space="PSUM"))
            rope_tmp = ph.enter_context(tc.tile_pool(name="rope_tmp", bufs=2))
            at = ph.enter_context(tc.tile_pool(name="at", bufs=3))
            psl = ph.enter_context(tc.tile_pool(name="psl", bufs=2, space="PSUM"))
            pss = ph.enter_context(tc.tile_pool(name="pss", bufs=1, space="PSUM"))
            pso = ph.enter_context(tc.tile_pool(name="pso", bufs=2, space="PSUM"))

            o28 = [o2_pool.tile([128, 2, CS], f8, name="o28", tag="o28")
                   for _ in range(4)]

            wqk_tiles = {}

            def issue_wqk(mt):
                w = wqk_pool.tile([128, 4, 4, 2, 128], f8, name="wqk", tag="wqk")
                nc.sync.dma_start(out=w, in_=wqk_d[mt])
                wqk_tiles[mt] = w

            def proj_chunks(mt):
                w = wqk_tiles.pop(mt)
                qt_t = qT_pool.tile([128, CS], bf16, tag="qT")
                kt_t = kT_pool.tile([128, HT], bf16, tag="kT")
                st = {}

                def emit_proj(ps, wbase, tok_lo, tok_hi):
                    cuts = sorted({tok_lo, tok_hi}
                                  | {c for c in (HH, 256, 640) if tok_lo < c < tok_hi})
                    chunks = list(zip(cuts[:-1], cuts[1:]))
                    for cn, (c0, c1) in enumerate(chunks):
                        reg = ps[:, c0 - tok_lo:c1 - tok_lo]
                        for ti, term in enumerate(qkv_terms):
                            hl = term[2]
                            for p in range(4):
                                nc.tensor.matmul(
                                    reg,
                                    lhsT=w[:, wbase + hl, p, :, :],
                                    rhs=ytok(term, c0, c1)(p),
                                    start=(ti == 0 and p == 0 and cn == 0),
                                    stop=(ti == len(qkv_terms) - 1 and p == 3),
                                    perf_mode=DR)

                def c0():  # q projection
                    ps = psb.tile([128, CS], f32, tag="psqk")
                    emit_proj(ps, 0, WIN, HT)
                    qb = rope_tmp.tile([128, CS], bf16, tag="ropesrc")
                    nc.scalar.activation(out=qb, in_=ps, func=AF.Identity,
                                         bias=bqk_sb[:, mt:mt + 1], scale=1.0)
                    st["qb"] = qb

                def c1():  # q rope
                    qb = st["qb"]
                    pr = psb.tile([128, 512], f32, tag="psqk")
                    nc.tensor.matmul(pr, lhsT=pshuf, rhs=qb, start=True, stop=True)
                    u = rope_tmp.tile([128, HT], bf16, tag="ropeu")
                    nc.vector.tensor_mul(out=u[:, :CS], in0=qb, in1=cosq)
                    t1 = rope_tmp.tile([128, 512], bf16, tag="ropet")
                    nc.vector.tensor_mul(out=t1, in0=pr, in1=msinq)
                    nc.vector.tensor_add(out=qt_t, in0=u[:, :CS], in1=t1)

                def c2():  # k projection half 0
                    kb = rope_tmp.tile([128, HT], bf16, tag="ropesrck")
                    st["kb"] = kb
                    ps = psb.tile([128, 384], f32, tag="psqk")
                    emit_proj(ps, 2, 0, 384)
                    nc.scalar.activation(out=kb[:, 0:384], in_=ps, func=AF.Identity,
                                         bias=bqk_sb[:, 8 + mt:9 + mt], scale=1.0)

                def c3():  # k projection half 1 + k rope
                    kb = st["kb"]
                    ps = psb.tile([128, 384], f32, tag="psqk")
                    emit_proj(ps, 2, 384, HT)
                    nc.scalar.activation(out=kb[:, 384:768], in_=ps, func=AF.Identity,
                                         bias=bqk_sb[:, 8 + mt:9 + mt], scale=1.0)
                    u = rope_tmp.tile([128, HT], bf16, tag="ropeu")
                    nc.vector.tensor_mul(out=u, in0=kb, in1=cosk)
                    for c in range(2):
                        w_ = 512 if c == 0 else 256
                        sl_ = slice(c * 512, c * 512 + w_)
                        pr = psb.tile([128, 512], f32, tag="psqk")
                        nc.tensor.matmul(pr[:, :w_], lhsT=pshuf, rhs=kb[:, sl_],
                                         start=True, stop=True)
                        t1 = rope_tmp.tile([128, 512], bf16, tag="ropet")
                        nc.vector.tensor_mul(out=t1[:, :w_], in0=pr[:, :w_],
                                             in1=msink[:, sl_])
                        nc.vector.tensor_add(out=kt_t[:, sl_], in0=u[:, sl_],
                                             in1=t1[:, :w_])

                qT.append(qt_t)
                kT.append(kt_t)
                return [c0, c1, c2, c3]

            def attn_front(mt, qt):
                """logits (PE) + exp (Act) + mask-mult (DVE) -> ET."""
                ps_l2 = []
                for hh in range(2):
                    hr = hh * 64
                    ps_l = psl.tile([128, 384], f32, tag="psl")
                    for j in range(3):
                        nc.tensor.matmul(
                            ps_l[:, j * 128:(j + 1) * 128],
                            lhsT=kT[mt][hr:hr + 64, (qt + j) * 128:(qt + j + 1) * 128],
                            rhs=qT[mt][hr:hr + 64, qt * 128:(qt + 1) * 128],
                            start=(j == 0), stop=(j == 2))
                    ps_l2.append(ps_l)
                Eb = at.tile([128, 2, 384], bf16, tag="Eb")
                for hh in range(2):
                    nc.scalar.activation(out=Eb[:, hh, :], in_=ps_l2[hh], func=AF.Exp,
                                         scale=float(HD) ** -0.5)
                ET = at.tile([128, 2, 384], bf16, tag="ET")
                for hh in range(2):
                    nc.vector.tensor_mul(out=ET[:, hh, :], in0=Eb[:, hh, :],
                                         in1=masks[qt])
                return (mt, qt, ET)

            def attn_back(ctx):
                """sums + AV (PE), then normalize into o28 (DVE)."""
                mt, qt, ET = ctx
                ps_s = pss.tile([128, 128], f32, tag="pss")
                ps_o = pso.tile([128, 128], f32, tag="pso")
                for hh in range(2):
                    hr = hh * 64
                    for j in range(3):
                        nc.tensor.matmul(ps_s[hr:hr + 64, :], lhsT=ones64,
                                         rhs=ET[:, hh, j * 128:(j + 1) * 128],
                                         start=(j == 0), stop=(j == 2))
                for hh in range(2):
                    hr = hh * 64
                    h = 2 * mt + hh
                    for j in range(3):
                        nc.tensor.matmul(ps_o[hr:hr + 64, :],
                                         lhsT=v_bf[qt + j][:, h * 64:h * 64 + 64],
                                         rhs=ET[:, hh, j * 128:(j + 1) * 128],
                                         start=(j == 0), stop=(j == 2))
                rec = at.tile([128, 128], f32, tag="rec")
                nc.vector.reciprocal(out=rec, in_=ps_s)
                nc.vector.tensor_mul(out=o28[mt // 2][:, mt % 2, qt * 128:(qt + 1) * 128],
                                     in0=ps_o, in1=rec)

            x2_list = [None] * NQT
            mv2_list = [None] * NQT
            y2_list = [None] * NQT

            def outproj_chunk(qt):
                def f():
                    x2 = x2_pool.tile([128, D], bf16, tag="x2")
                    for half in range(2):
                        ps = psb.tile([128, 512], f32, tag="psqk")
                        for cn in range(2):
                            reg = ps[:, cn * 256:(cn + 1) * 256]
                            for p in range(4):
                                nc.tensor.matmul(
                                    reg,
                                    lhsT=o28[p][:, :, qt * 128:(qt + 1) * 128],
                                    rhs=wo8[:, p, :,
                                            half * 512 + cn * 256:half * 512 + (cn + 1) * 256],
                                    start=(p == 0 and cn == 0), stop=(p == 3),
                                    perf_mode=DR)
                        sl = slice(half * 512, (half + 1) * 512)
                        nc.vector.tensor_add(out=x2[:, sl], in0=ps,
                                             in1=x_tiles[2 + qt][:, sl])
                    x2_list[qt] = x2
                    mv2_list[qt] = ln_stats(x2, ln_tmp2)
                return f

            def lnfin_chunk(qt):
                def f():
                    y2 = y2_pool.tile([128, D], bf16, tag="y2")
                    ln_norm(x2_list[qt], y2, mv2_list[qt], ln_tmp2)
                    y2_list[qt] = y2
                return f

            issue_wqk(0)
            issue_wqk(1)
            chunks = proj_chunks(0)
            for c in chunks:
                c()
            fill_plan = {
                (7, 1): [outproj_chunk(0), lnfin_chunk(0)],
                (7, 2): [outproj_chunk(1), lnfin_chunk(1)],
                (7, 3): [outproj_chunk(2), lnfin_chunk(2)],
            }
            ctx = None
            for mt in range(8):
                if mt + 2 < 8:
                    issue_wqk(mt + 2)
                if mt + 1 < 8:
                    nxt = proj_chunks(mt + 1)
                for qt in range(NQT):
                    nctx = attn_front(mt, qt)
                    if ctx is not None:
                        attn_back(ctx)
                    if mt + 1 < 8:
                        nxt[qt]()
                    else:
                        for fl in fill_plan.get((mt, qt), []):
                            fl()
                    ctx = nctx
            attn_back(ctx)
            outproj_chunk(NQT - 1)()
            lnfin_chunk(3)()

        qkv_scope.close()

        # ====== phases D: y2^T triplet interleaved with MLP gate/up ======
        H8 = []
        H8s = []
        with ExitStack() as ph:
            pst2 = ph.enter_context(tc.tile_pool(name="pst2", bufs=4, space="PSUM"))
            wgu_pool = ph.enter_context(tc.tile_pool(name="wgu", bufs=3))
            psg = ph.enter_context(tc.tile_pool(name="psg", bufs=4, space="PSUM"))
            gu_tmp = ph.enter_context(tc.tile_pool(name="gu_tmp", bufs=4))

            y2T8 = [y2T_pool.tile([128, 2, CS], f8, name="y2T8", tag="y2T8")
                    for _ in range(4)]
            y2T8s = [y2T_pool.tile([128, 2, CS], f8, name="y2T8s", tag="y2T8s")
                     for _ in range(4)]
            y2Tr8 = [y2T_pool.tile([128, 2, CS], f8, name="y2Tr8", tag="y2Tr8")
                     for _ in range(4)] if GU_THIRD else None
            gu_terms = [(y2T8, 0), (y2T8s, 1)] + ([(y2Tr8, 0)] if GU_THIRD else [])
            nterm = len(gu_terms)

            for pair in range(16):
                H8.append(hh_pool.tile([128, 2, CS], f8, name="H8", tag="hh"))
                H8s.append(hh_pool.tile([128, 2, CS], f8, name="H8s", tag="hhs"))

            def y2t_wave(dts):
                pts = {}
                for dtl in dts:
                    pts[dtl] = pst2.tile([128, 4, 128], bf16, name="pt2",
                                         tag="pst2b")
                for qt in range(NQT):
                    for dtl in dts:
                        nc.tensor.transpose(pts[dtl][:, qt, :],
                                            y2_list[qt][:, dtl * 128:(dtl + 1) * 128],
                                            identb)
                for dtl in dts:
                    pt = pts[dtl]
                    dst_hi = y2T8[dtl // 2][:, dtl % 2, :]
                    nc.scalar.activation(out=dst_hi, in_=pt, func=AF.Identity)
                    nc.gpsimd.tensor_scalar_mul(out=y2T8s[dtl // 2][:, dtl % 2, :],
                                                in0=dst_hi, scalar1=1.0 / 16)
                    if GU_THIRD:
                        nc.vector.tensor_tensor(out=y2Tr8[dtl // 2][:, dtl % 2, :],
                                                in0=pt, in1=dst_hi, op=OP.subtract)

            wgu_tiles = {}
            wd_tiles = {}

            def issue_wd(pair):
                w = wd_pool.tile([128, 2, 2, 1024], f8, name="wd", tag="wd")
                eng = nc.sync if pair % 2 == 0 else nc.gpsimd
                eng.dma_start(out=w, in_=wd_d[pair])
                wd_tiles[pair] = w

            def gu_mt_pass(mt, ps_pair, p):
                """One k-pair accumulation pass of gate+up for f-block mt."""
                w = wgu_tiles[mt]
                for gi in range(2):
                    ps = ps_pair[gi]
                    for cn in range(2):
                        reg = ps[:, cn * 256:(cn + 1) * 256]
                        for ti, (act, hl) in enumerate(gu_terms):
                            last = (p == 3 and ti == nterm - 1)
                            nc.tensor.matmul(
                                reg,
                                lhsT=w[:, gi, hl, p, :, :],
                                rhs=act[p][:, :, cn * 256:(cn + 1) * 256],
                                start=(p == 0 and ti == 0 and cn == 0),
                                stop=(last and not (has_bg and gi == 0)),
                                perf_mode=DR)

            def gu_mt_finish(mt, ps_pair):
                if has_bg:
                    for cn in range(2):
                        nc.tensor.matmul(
                            ps_pair[0][:, cn * 256:(cn + 1) * 256],
                            lhsT=bg_sb[:, mt * 128:(mt + 1) * 128],
                            rhs=ones_row[:, cn * 256:(cn + 1) * 256],
                            start=False, stop=True)
                U = gu_tmp.tile([128, CS], bf16, tag="U")
                nc.scalar.activation(out=U, in_=ps_pair[1], func=AF.Silu,
                                     bias=bu_sb[:, mt:mt + 1], scale=1.0)
                h8_dst = H8[mt // 2][:, mt % 2, :]
                nc.vector.tensor_mul(out=h8_dst, in0=ps_pair[0], in1=U)
                nc.gpsimd.tensor_scalar_mul(out=H8s[mt // 2][:, mt % 2, :],
                                            in0=h8_dst, scalar1=0.125)

            def new_gu_ps():
                return [psg.tile([128, CS], f32, name="psgu", tag="psgu")
                        for _ in range(2)]

            def issue_wgu(mt):
                pool = wgu0_pool if mt < 2 else wgu_pool
                w = pool.tile([128, 2, 2, 4, 2, 128], f8, name="wgu", tag="wgu")
                eng = nc.sync if mt % 2 == 0 else nc.gpsimd
                eng.dma_start(out=w, in_=wgu_d[mt])
                wgu_tiles[mt] = w

            # mt 0/1: pair passes interleaved with the y2T wave production so
            # PE stays fed while the transposes/copies stream out
            issue_wgu(0)
            issue_wgu(1)
            ps0, ps1 = new_gu_ps(), new_gu_ps()
            y2t_wave([0, 1, 2, 3])
            for p in (0, 1):
                gu_mt_pass(0, ps0, p)
                gu_mt_pass(1, ps1, p)
            y2t_wave([4, 5, 6, 7])
            for p in (2, 3):
                gu_mt_pass(0, ps0, p)
                gu_mt_pass(1, ps1, p)
            gu_mt_finish(0, ps0)
            gu_mt_finish(1, ps1)
            wgu_tiles.pop(0)
            wgu_tiles.pop(1)

            issue_wgu(2)
            for mt in range(2, 32):
                if mt + 1 < 32:
                    issue_wgu(mt + 1)
                if mt in (19, 21, 23, 25):
                    issue_wd((mt - 19) // 2)
                w = wgu_tiles[mt]
                psm = new_gu_ps()
                for p in range(4):
                    gu_mt_pass(mt, psm, p)
                gu_mt_finish(mt, psm)
                wgu_tiles.pop(mt)

        mid_scope.close()

        # ====== phase E: down proj (x4 weights) + residual + store ======
        with ExitStack() as ph:
            psd = ph.enter_context(tc.tile_pool(name="psd", bufs=8, space="PSUM"))
            out_pool = ph.enter_context(tc.tile_pool(name="outp", bufs=4))
            dn_tmp = ph.enter_context(tc.tile_pool(name="dn_tmp", bufs=4))

            ps_d = [psd.tile([128, 512], f32, name="psd", tag="psd") for _ in range(8)]
            dn_terms = [(H8, 0), (H8s, 1)]

            def dn_finish(tt):
                ot = out_pool.tile([128, D], f32, name="outp", tag="outp")
                for ch3 in range(2):
                    sl = slice(ch3 * 512, (ch3 + 1) * 512)
                    pd = ps_d[tt * 2 + ch3]
                    if has_bd:
                        # bias pre-scaled x4 on host to match the x4 weights
                        nc.tensor.matmul(pd[:, 0:256], lhsT=ones_row[:, 0:128],
                                         rhs=bd_sb[:, ch3 * 512:ch3 * 512 + 256],
                                         start=False, stop=True)
                        nc.tensor.matmul(pd[:, 256:512], lhsT=ones_row[:, 0:128],
                                         rhs=bd_sb[:, ch3 * 512 + 256:(ch3 + 1) * 512],
                                         start=False, stop=True)
                    tmp = dn_tmp.tile([128, 512], f32, name="dntmp", tag="dntmp")
                    nc.scalar.activation(out=tmp, in_=pd, func=AF.Identity,
                                         scale=0.25)
                    nc.vector.tensor_add(out=ot[:, sl], in0=tmp,
                                         in1=x2_list[tt][:, sl])
                eng2 = nc.sync if tt % 2 == 0 else nc.gpsimd
                eng2.dma_start(out=out_d[tt * 128:(tt + 1) * 128, :], in_=ot)

            for pair in range(16):
                if pair + 4 < 16:
                    issue_wd(pair + 4)
                w = wd_tiles.pop(pair)
                for tt in range(NQT):
                    for ti, (act, hl) in enumerate(dn_terms):
                        for cn in range(4):
                            reg = ps_d[tt * 2 + cn // 2][:, (cn % 2) * 256:
                                                         (cn % 2 + 1) * 256]
                            nc.tensor.matmul(
                                reg,
                                lhsT=act[pair][:, :, tt * 128:(tt + 1) * 128],
                                rhs=w[:, hl, :, cn * 256:(cn + 1) * 256],
                                start=(pair == 0 and ti == 0 and cn % 2 == 0),
                                stop=(pair == 15 and ti == 1 and not has_bd),
                                perf_mode=DR)
                    if pair == 15:
                        dn_finish(tt)
        mlp_scope.close()
        wqkp_scope.close()

    nc.compile()
    return nc


def prep_inputs(x, w_qkv, w_out, g1, b1, g2, b2, w_gate, b_gate, w_up, b_up,
                w_down, b_down):
    """Host-side: fold LN params, fp8-split weights, pre-tile, build per-core
    tensors."""
    import ml_dtypes
    f32 = np.float32
    bf16 = ml_dtypes.bfloat16
    f8 = ml_dtypes.float8_e4m3

    def split8(w, s=16.0):
        hi = w.astype(f8)
        lo = ((w - hi.astype(f32)) * s).astype(f8)
        return hi, lo

    wqkv_f = (w_qkv * g1[:, None]).astype(f32)
    bqkv = (b1 @ w_qkv).astype(f32)

    def qk_tile(w):  # [D, 1024] -> [mt, p, pair, i, m] fp8 pieces
        hi, lo = split8(w)
        t = lambda a: np.ascontiguousarray(
            a.reshape(4, 2, 128, 8, 128).transpose(3, 2, 0, 1, 4))
        return t(hi), t(lo)

    qhi, qlo = qk_tile(wqkv_f[:, :D])
    khi, klo = qk_tile(wqkv_f[:, D:2 * D])
    wqk = np.ascontiguousarray(
        np.stack([qhi, qlo, khi, klo], axis=2))  # [8,128,4,4,2,128]

    def mv_tile(w):  # [D, 1024] -> [p, pair, i, n]
        return w.reshape(4, 2, 128, 1024).transpose(2, 0, 1, 3)

    vhi, vlo = split8(wqkv_f[:, 2 * D:])
    wv = np.ascontiguousarray(np.stack([mv_tile(vhi), mv_tile(vlo)], axis=1))
    wo = np.ascontiguousarray(mv_tile(w_out.astype(f32).astype(f8)))

    def gu_tile(w):  # [D, F] -> [mt, p, hi/lo, pair, i, m]
        hi, lo = split8(w)
        t = lambda a: a.reshape(4, 2, 128, 32, 128).transpose(3, 2, 0, 1, 4)
        return np.stack([t(hi), t(lo)], axis=2)  # [32,128,2,4,2,128]

    wg_f = (w_gate * g2[:, None]).astype(f32)
    wu_f = (w_up * g2[:, None]).astype(f32)
    wgu = np.ascontiguousarray(
        np.stack([gu_tile(wg_f), gu_tile(wu_f)], axis=2))  # [32,128,2,2,4,2,128]

    wd_f = w_down.astype(f32)
    wd_hi = (4.0 * wd_f).astype(f8)
    wd_lo = (32.0 * (wd_f - wd_hi.astype(f32) / 4.0)).astype(f8)
    t_wd = lambda a: a.reshape(16, 2, 128, 1024).transpose(0, 2, 1, 3)
    wd = np.ascontiguousarray(np.stack([t_wd(wd_hi), t_wd(wd_lo)], axis=2))

    bqk_pt = bqkv[:2048].reshape(16, 128).T                       # [p, t]
    bu_pt = (b_up + b2 @ w_up).astype(f32).reshape(32, 128).T
    cbf = np.ascontiguousarray(
        np.concatenate([bqk_pt, bu_pt], axis=1)).astype(f32)      # [128, 48]

    bg_row = (b_gate + b2 @ w_gate).astype(f32).reshape(1, F).astype(bf16)
    bv_row = bqkv[2048:].reshape(1, D).astype(bf16)
    bd_row = (4.0 * b_down).reshape(1, D).astype(bf16)

    # rotate-half permutation (sign folded into sin tables)
    pshuf = np.zeros((128, 128), f32)
    for m in range(128):
        base = (m // 64) * 64
        r = m % 64
        sig = base + (r + 32) % 64
        pshuf[sig, m] = 1.0
    pshuf = pshuf.astype(bf16)

    half = HD // 2
    inv_freq = 1.0 / (10000.0 ** (np.arange(half, dtype=np.float64) / half))

    def rope_tables(pos):
        t = np.maximum(pos, 0).astype(np.float64)
        freqs = np.outer(t, inv_freq)
        emb = np.concatenate([freqs, freqs], 1)
        c = np.cos(emb).T.astype(f32)
        s = np.sin(emb).T.astype(f32)
        ms = s.copy()
        ms[:32] = -ms[:32]
        return (np.ascontiguousarray(np.vstack([c, c])),
                np.ascontiguousarray(np.vstack([ms, ms])))

    common = {"wqk": wqk, "wv": wv, "wo": wo, "wgu": wgu, "wd": wd,
              "bv": bv_row, "bd": bd_row, "bg": bg_row, "cbf": cbf}

    in_maps = []
    for c in range(NCORES):
        b, chunk = c // CH, c % CH
        q0 = chunk * CS
        lo = q0 - WIN
        xh = np.zeros((HT, D), f32)
        src_lo = max(0, lo)
        xh[src_lo - lo:] = x[b, src_lo:q0 + CS]
        xh = xh.astype(bf16)
        pos_k = np.arange(lo, q0 + CS)
        cosk_a, sink_a = rope_tables(pos_k)
        cosq_a = np.ascontiguousarray(cosk_a[:, WIN:]).astype(bf16)
        sinq_a = np.ascontiguousarray(sink_a[:, WIN:]).astype(bf16)
        # transposed multiplicative mask [r, qt, j, c]:
        #   key j_g = lo + (qt+j)*128 + r ; query i = q0 + qt*128 + c
        r_i = np.arange(128)[:, None, None, None]
        qt_i = np.arange(NQT)[None, :, None, None]
        j_i = np.arange(3)[None, None, :, None]
        c_i = np.arange(128)[None, None, None, :]
        jg = lo + (qt_i + j_i) * 128 + r_i
        gi = q0 + qt_i * 128 + c_i
        valid = (jg <= gi) & (gi - jg <= WIN) & (jg >= 0)
        maskT = valid.astype(f32).reshape(128, NQT * 3 * 128).astype(bf16)
        cbb = np.concatenate(
            [cosq_a, sinq_a, cosk_a.astype(bf16), sink_a.astype(bf16),
             maskT, pshuf], axis=1)
        in_maps.append(dict(common, xh=xh, cbb=np.ascontiguousarray(cbb)))
    return in_maps


_PROG = {}


def kernel(**inputs):
    from concourse.bass_utils import run_bass_kernel_spmd

    inputs = {k: np.asarray(v, dtype=np.float32) for k, v in inputs.items()}
    in_maps = prep_inputs(**inputs)
    flags = (bool(np.any(inputs["b1"] @ inputs["w_qkv"][:, 2048:])),
             bool(np.any(inputs["b_gate"] + inputs["b2"] @ inputs["w_gate"])),
             bool(np.any(inputs["b_down"])))
    if flags not in _PROG:
        _PROG[flags] = build_program(has_bv=flags[0], has_bg=flags[1],
                                     has_bd=flags[2])
    nc = _PROG[flags]
    res = run_bass_kernel_spmd(nc, in_maps, core_ids=list(range(NCORES)))
    out = np.zeros((B, S, D), np.float32)
    for c in range(NCORES):
        b, chunk = c // CH, c % CH
        out[b, chunk * CS:(chunk + 1) * CS] = res.results[c]["out"]
    return out


# revision 27
# speedup vs baseline: 1.2487x; 1.0066x over previous
"""Trainium2 Bass kernel for AdvancedTransformerEncoderBlock (fp8 DoubleRow).

Sharding: token-parallel across 8 cores (B=2 x 4 seq chunks of 512), each core
recomputes a 256-token K/V halo -> zero collectives.

Precision plan (validated vs fp32 reference, rel_err ~= 0.015):
  - qkv proj:   fp8e4 DoubleRow, weights split hi+lo(x16), activation split
                hi + hi/16 + residual  (3 passes, 4x per-pass speedup)
  - attention:  bf16 (transposed-logits flow: logits land [keys, queries] in
                PSUM; exp on Act; band mask folded into the PSUM->SBUF copy as
                a 0/1 multiply; softmax sums via ones[128,64] matmul so the
                per-query denominators arrive broadcast across partitions;
                normalize folded into the o2 copy)
  - out proj:   fp8e4 DoubleRow single-pass (o2/wo plain fp8)
  - gate/up:    like qkv (3 passes)
  - down proj:  weights split fp8(4w) + fp8(32*res), H plain fp8 + H/8 copy;
                the 4x weight prescale (keeps wd out of fp8 subnormals) is
                undone by a 0.25 scale folded into the PSUM->SBUF copy
PSUM accumulation stays fp32, residual stream stays fp32.
RoPE rotate-half runs as a PE permutation matmul.
Attention runs one query-tile ahead on logits so exp/mask latency hides under
sums/AV of the previous tile plus the interleaved projection fillers.
"""

import numpy as np

B, S, D, F, H, HD = 2, 2048, 1024, 4096, 16, 64
WIN = 256
NCORES = 8
CH = 4           # chunks per batch
CS = S // CH     # 512 tokens per chunk (queries)
HT = CS + WIN    # 768 tokens incl. halo (keys/values)
NQT = CS // 128  # 4 query tiles
EPS = 1e-5
QKV_THIRD = True   # include activation-residual pass in qkv proj
GU_THIRD = True    # include activation-residual pass in gate/up


def build_program(has_bv=False, has_bg=False, has_bd=False):
    import concourse.bass as bass
    import concourse.bacc as bacc_mod
    import concourse.tile as tile
    import concourse.mybir as mybir
    from concourse.masks import make_identity
    from contextlib import ExitStack

    dt = mybir.dt
    f32, bf16, f8 = dt.float32, dt.bfloat16, dt.float8e4
    AF = mybir.ActivationFunctionType
    OP = mybir.AluOpType
    DR = mybir.MatmulPerfMode.DoubleRow

    nc = bacc_mod.Bacc()
    Pf = lambda name, shape: nc.declare_dram_parameter(name, list(shape), f32, isOutput=False)
    Pb = lambda name, shape: nc.declare_dram_parameter(name, list(shape), bf16, isOutput=False)
    P8 = lambda name, shape: nc.declare_dram_parameter(name, list(shape), f8, isOutput=False)

    xh_d = Pb("xh", (HT, D))
    wqk_d = P8("wqk", (8, 128, 4, 4, 2, 128))   # [mt][p][qhi,qlo,khi,klo][pair][i][m]
    wv_d = P8("wv", (128, 2, 4, 2, 1024))       # [p][hi/lo][pair][i][n]
    wo_d = P8("wo", (128, 4, 2, 1024))          # [p][pair][i][n]
    wgu_d = P8("wgu", (32, 128, 2, 2, 4, 2, 128))  # [mt][p][g/u][hi/lo][pair][i][m]
    wd_d = P8("wd", (16, 128, 2, 2, 1024))      # [pair][p][hi/lo][i][n]
    bv_d = Pb("bv", (1, D))
    bd_d = Pb("bd", (1, D))
    bg_d = Pb("bg", (1, F))
    cbf_d = Pf("cbf", (128, 48))                # bqk [:,0:16], bu [:,16:48]
    cbb_d = Pb("cbb", (128, 4224))
    out_d = nc.declare_dram_parameter("out", [CS, D], f32, isOutput=True)

    with tile.TileContext(nc) as tc, ExitStack() as top:
        const = top.enter_context(tc.tile_pool(name="const", bufs=1))

        # x tiles first: their DMAs head the queue so LN/transposes start early
        x_pool = top.enter_context(tc.tile_pool(name="x", bufs=6))
        x_tiles = []
        for tt in range(6):
            xt = x_pool.tile([128, D], bf16, tag="xt")
            eng = nc.sync if tt % 2 == 0 else nc.gpsimd
            if tt == 0:
                eng.dma_start(out=xt[:, 0:512], in_=xh_d[0:128, 0:512])
                eng.dma_start(out=xt[:, 512:1024], in_=xh_d[0:128, 512:1024])
            else:
                eng.dma_start(out=xt, in_=xh_d[tt * 128:(tt + 1) * 128, :])
            x_tiles.append(xt)

        # ---- constants ----
        cbf = const.tile([128, 48], f32, tag="cbf")
        nc.sync.dma_start(out=cbf, in_=cbf_d[:, :])
        cbb = const.tile([128, 4224], bf16, tag="cbb")
        nc.gpsimd.dma_start(out=cbb, in_=cbb_d[:, :])
        bqk_sb = cbf[:, 0:16]
        bu_sb = cbf[:, 16:48]
        cosq = cbb[:, 0:512]
        msinq = cbb[:, 512:1024]
        cosk = cbb[:, 1024:1792]
        msink = cbb[:, 1792:2560]
        masks = [cbb[:, 2560 + qt * 384:2560 + (qt + 1) * 384] for qt in range(NQT)]
        pshuf = cbb[:, 4096:4224]

        identb = const.tile([128, 128], bf16, tag="identb")
        make_identity(nc, identb)
        ones64 = const.tile([128, 64], bf16, tag="ones64")
        nc.vector.memset(ones64, 1.0)
        ones_row = const.tile([1, 512], bf16, tag="ones_row")
        nc.vector.memset(ones_row, 1.0)
        eps_t = const.tile([128, 1], f32, tag="eps")
        nc.vector.memset(eps_t, EPS)
        if has_bv:
            bv_sb = const.tile([1, D], bf16, tag="bv")
            nc.sync.dma_start(out=bv_sb, in_=bv_d[:, :])
        if has_bd:
            bd_sb = const.tile([1, D], bf16, tag="bd")
            nc.sync.dma_start(out=bd_sb, in_=bd_d[:, :])
        if has_bg:
            bg_sb = const.tile([1, F], bf16, tag="bg")
            nc.sync.dma_start(out=bg_sb, in_=bg_d[:, :])

        # ---- persistent activation pools ----
        x2_pool = top.enter_context(tc.tile_pool(name="x2", bufs=4))
        y2T_pool = top.enter_context(tc.tile_pool(name="y2T", bufs=4))
        o2_pool = top.enter_context(tc.tile_pool(name="o2", bufs=4))

        def ln_stats(src, tmp_pool):
            stats = tmp_pool.tile([128, 2, 6], f32, tag="lnstats")
            mv = tmp_pool.tile([128, 2], f32, tag="lnmv")
            for sg in range(2):
                nc.vector.bn_stats(out=stats[:, sg, :], in_=src[:, sg * 512:(sg + 1) * 512])
            nc.vector.bn_aggr(out=mv, in_=stats)
            return mv

        def ln_norm(src, dst, mv, tmp_pool):
            rs = tmp_pool.tile([128, 1], f32, tag="lnrs")
            nc.scalar.activation(out=rs, in_=mv[:, 1:2], func=AF.Sqrt,
                                 bias=eps_t, scale=1.0)
            nc.vector.reciprocal(out=rs, in_=rs)
            nb = tmp_pool.tile([128, 1], f32, tag="lnnb")
            nc.vector.tensor_scalar(out=nb, in0=mv[:, 0:1], scalar1=rs,
                                    scalar2=-1.0, op0=OP.mult, op1=OP.mult)
            nc.scalar.activation(out=dst, in_=src, func=AF.Identity,
                                 bias=nb, scale=rs)

        def layernorm(src, dst, tmp_pool):
            ln_norm(src, dst, ln_stats(src, tmp_pool), tmp_pool)

        # late-lived pools opened early for LIFO stack order (no instructions
        # are emitted at open time); wqk gets fresh persistent space so its
        # streaming DMAs never wait on freed-pool readers
        wqkp_scope = ExitStack()
        wqk_pool = wqkp_scope.enter_context(tc.tile_pool(name="wqkp", bufs=3))
        mlp_scope = ExitStack()
        hh_pool = mlp_scope.enter_context(tc.tile_pool(name="hh", bufs=16))
        wd_pool = mlp_scope.enter_context(tc.tile_pool(name="wd", bufs=4))
        wgu0_pool = mlp_scope.enter_context(tc.tile_pool(name="wgu0", bufs=2))

        # mid-lived: x2-LN workspace + y2 (read by the late y2T transposes)
        mid_scope = ExitStack()
        ln_tmp2 = mid_scope.enter_context(tc.tile_pool(name="ln_tmp2", bufs=3))
        y2_pool = mid_scope.enter_context(tc.tile_pool(name="y2", bufs=4))

        qkv_scope = ExitStack()
        yT_pool = qkv_scope.enter_context(tc.tile_pool(name="yT", bufs=4))
        wo_pool = qkv_scope.enter_context(tc.tile_pool(name="wo", bufs=1))
        wo8 = wo_pool.tile([128, 4, 2, 1024], f8, tag="wo8")
        qT_pool = qkv_scope.enter_context(tc.tile_pool(name="qT", bufs=3))
        kT_pool = qkv_scope.enter_context(tc.tile_pool(name="kT", bufs=3))
        vb_pool = qkv_scope.enter_context(tc.tile_pool(name="vb", bufs=6))

        # v-proj weights: DMAs queued ahead of everything but x (phase B is
        # the first weight consumer); wo follows on the same queue
        vw_scope = ExitStack()
        wv_pool = vw_scope.enter_context(tc.tile_pool(name="wv", bufs=1))
        wv8 = wv_pool.tile([128, 2, 4, 2, 1024], f8, tag="wv8")
        for p_ in range(4):
            eng = nc.sync if p_ % 2 == 0 else nc.gpsimd
            eng.dma_start(out=wv8[:, :, p_, :, :], in_=wv_d[:, :, p_, :, :])
        nc.gpsimd.dma_start(out=wo8, in_=wo_d[:, :, :, :])

        # y^T pair tiles split by token halves so consumers start after the
        # first three LN outputs: a = tokens 0:384, b = 384:768
        HH = HT // 2
        yT8a = [yT_pool.tile([128, 2, HH], f8, name="yT8a", tag="yT8a") for _ in range(4)]
        yT8b = [yT_pool.tile([128, 2, HH], f8, name="yT8b", tag="yT8b") for _ in range(4)]
        yT8sa = [yT_pool.tile([128, 2, HH], f8, name="yT8sa", tag="yT8sa") for _ in range(4)]
        yT8sb = [yT_pool.tile([128, 2, HH], f8, name="yT8sb", tag="yT8sb") for _ in range(4)]
        yTr8a = [yT_pool.tile([128, 2, HH], f8, name="yTr8a", tag="yTr8a")
                 for _ in range(4)] if QKV_THIRD else None
        yTr8b = [yT_pool.tile([128, 2, HH], f8, name="yTr8b", tag="yTr8b")
                 for _ in range(4)] if QKV_THIRD else None

        # =========== phase A: LN1 -> y -> y^T fp8 triplet ===========
        with ExitStack() as ph:
            ln_tmp = ph.enter_context(tc.tile_pool(name="ln_tmp", bufs=6))
            y_pool = ph.enter_context(tc.tile_pool(name="y", bufs=6))
            pst = ph.enter_context(tc.tile_pool(name="pst", bufs=8, space="PSUM"))

            ys = []
            for tt in range(6):
                y = y_pool.tile([128, D], bf16, tag="y")
                layernorm(x_tiles[tt], y, ln_tmp)
                ys.append(y)
            # tt-outer with half-granular copies: the a-half (tokens 0:384)
            # ships as soon as the first three LN outputs exist
            pts = [pst.tile([128, 6, 128], bf16, name="pt", tag="pst")
                   for _ in range(8)]
            for half, (hi_l, s_l, r_l) in enumerate(
                    [(yT8a, yT8sa, yTr8a), (yT8b, yT8sb, yTr8b)]):
                for tt in range(half * 3, half * 3 + 3):
                    for dtl in range(8):
                        nc.tensor.transpose(pts[dtl][:, tt, :],
                                            ys[tt][:, dtl * 128:(dtl + 1) * 128],
                                            identb)
                for dtl in range(8):
                    pt = pts[dtl][:, half * 3:half * 3 + 3, :]
                    dst_hi = hi_l[dtl // 2][:, dtl % 2, :]
                    nc.scalar.activation(out=dst_hi, in_=pt, func=AF.Identity)
                    nc.gpsimd.tensor_scalar_mul(out=s_l[dtl // 2][:, dtl % 2, :],
                                                in0=dst_hi, scalar1=1.0 / 16)
                    if QKV_THIRD:
                        nc.vector.tensor_tensor(out=r_l[dtl // 2][:, dtl % 2, :],
                                                in0=pt, in1=dst_hi, op=OP.subtract)

        qkv_terms = [(yT8a, yT8b, 0), (yT8sa, yT8sb, 1)] + \
            ([(yTr8a, yTr8b, 0)] if QKV_THIRD else [])

        def ytok(term, lo, hi):
            """AP for token range [lo, hi) of a qkv term (within one half)."""
            a_l, b_l, _ = term
            if hi <= HH:
                return lambda p: a_l[p][:, :, lo:hi]
            assert lo >= HH
            return lambda p: b_l[p][:, :, lo - HH:hi - HH]

        # =========== phase B: v projection (fp8 DR, pair-outer) ===========
        v_bf = []
        with ExitStack() as ph:
            psv = ph.enter_context(tc.tile_pool(name="psv", bufs=6, space="PSUM"))
            for tt in range(6):
                v_bf.append(vb_pool.tile([128, D], f8, name="vbf", tag="vbf"))
            nterm = len(qkv_terms)
            for chv in range(2):
                pv = [psv.tile([128, 512], f32, name="psv", tag="psv") for _ in range(6)]
                for ti, term in enumerate(qkv_terms):
                    hl = term[2]
                    for p in range(4):
                        for tt in range(6):
                            lsrc = ytok(term, tt * 128, (tt + 1) * 128)(p)
                            for cn in range(2):
                                reg = pv[tt][:, cn * 256:(cn + 1) * 256]
                                last = (p == 3 and ti == nterm - 1)
                                nc.tensor.matmul(
                                    reg,
                                    lhsT=lsrc,
                                    rhs=wv8[:, hl, p, :,
                                            chv * 512 + cn * 256:chv * 512 + (cn + 1) * 256],
                                    start=(ti == 0 and p == 0 and cn == 0),
                                    stop=(last and not has_bv), perf_mode=DR)
                if has_bv:
                    for tt in range(6):
                        for cn in range(2):
                            nc.tensor.matmul(pv[tt][:, cn * 256:(cn + 1) * 256],
                                             lhsT=ones_row[:, 0:128],
                                             rhs=bv_sb[:, chv * 512 + cn * 256:
                                                       chv * 512 + (cn + 1) * 256],
                                             start=False, stop=True)
                for tt in range(6):
                    sl = slice(chv * 512, (chv + 1) * 512)
                    if tt % 2 == 0:
                        nc.scalar.copy(out=v_bf[tt][:, sl], in_=pv[tt])
                    else:
                        nc.vector.tensor_copy(out=v_bf[tt][:, sl], in_=pv[tt])
        vw_scope.close()

        # ==== phase C: q/k proj + RoPE pipelined with attention (flow B) ====
        qT, kT = [], []
        with ExitStack() as ph:
            psb = ph.enter_context(tc.tile_pool(name="psb", bufs=3, space="PSUM"))
            rope_tmp = ph.enter_context(tc.tile_pool(name="rope_tmp", bufs=2))
            at = ph.enter_context(tc.tile_pool(name="at", bufs=3))
            psl = ph.enter_context(tc.tile_pool(name="psl", bufs=2, space="PSUM"))
            pss = ph.enter_context(tc.tile_pool(name="pss", bufs=1, space="PSUM"))
            pso = ph.enter_context(tc.tile_pool(name="pso", bufs=2, space="PSUM"))

            o28 = [o2_pool.tile([128, 2, CS], f8, name="o28", tag="o28")
                   for _ in range(4)]

            wqk_tiles = {}

            def issue_wqk(mt):
                w = wqk_pool.tile([128, 4, 4, 2, 128], f8, name="wqk", tag="wqk")
                nc.sync.dma_start(out=w, in_=wqk_d[mt])
                wqk_tiles[mt] = w

            def proj_chunks(mt):
                w = wqk_tiles.pop(mt)
                qt_t = qT_pool.tile([128, CS], bf16, tag="qT")
                kt_t = kT_pool.tile([128, HT], bf16, tag="kT")
                st = {}

                def emit_proj(ps, wbase, tok_lo, tok_hi):
                    cuts = sorted({tok_lo, tok_hi}
                                  | {c for c in (HH, 256, 640) if tok_lo < c < tok_hi})
                    chunks = list(zip(cuts[:-1], cuts[1:]))
                    for cn, (c0, c1) in enumerate(chunks):
                        reg = ps[:, c0 - tok_lo:c1 - tok_lo]
                        for ti, term in enumerate(qkv_terms):
                            hl = term[2]
                            for p in range(4):
                                nc.tensor.matmul(
                                    reg,
                                    lhsT=w[:, wbase + hl, p, :, :],
                                    rhs=ytok(term, c0, c1)(p),
                                    start=(ti == 0 and p == 0 and cn == 0),
                                    stop=(ti == len(qkv_terms) - 1 and p == 3),
                                    perf_mode=DR)

                def c0():  # q projection
                    ps = psb.tile([128, CS], f32, tag="psqk")
                    emit_proj(ps, 0, WIN, HT)
                    qb = rope_tmp.tile([128, CS], bf16, tag="ropesrc")
                    nc.scalar.activation(out=qb, in_=ps, func=AF.Identity,
                                         bias=bqk_sb[:, mt:mt + 1], scale=1.0)
                    st["qb"] = qb

                def c1():  # q rope
                    qb = st["qb"]
                    pr = psb.tile([128, 512], f32, tag="psqk")
                    nc.tensor.matmul(pr, lhsT=pshuf, rhs=qb, start=True, stop=True)
                    u = rope_tmp.tile([128, HT], bf16, tag="ropeu")
                    nc.vector.tensor_mul(out=u[:, :CS], in0=qb, in1=cosq)
                    t1 = rope_tmp.tile([128, 512], bf16, tag="ropet")
                    nc.vector.tensor_mul(out=t1, in0=pr, in1=msinq)
                    nc.vector.tensor_add(out=qt_t, in0=u[:, :CS], in1=t1)

                def c2():  # k projection half 0
                    kb = rope_tmp.tile([128, HT], bf16, tag="ropesrck")
                    st["kb"] = kb
                    ps = psb.tile([128, 384], f32, tag="psqk")
                    emit_proj(ps, 2, 0, 384)
                    nc.scalar.activation(out=kb[:, 0:384], in_=ps, func=AF.Identity,
                                         bias=bqk_sb[:, 8 + mt:9 + mt], scale=1.0)

                def c3():  # k projection half 1 + k rope
                    kb = st["kb"]
                    ps = psb.tile([128, 384], f32, tag="psqk")
                    emit_proj(ps, 2, 384, HT)
                    nc.scalar.activation(out=kb[:, 384:768], in_=ps, func=AF.Identity,
                                         bias=bqk_sb[:, 8 + mt:9 + mt], scale=1.0)
                    u = rope_tmp.tile([128, HT], bf16, tag="ropeu")
                    nc.vector.tensor_mul(out=u, in0=kb, in1=cosk)
                    for c in range(2):
                        w_ = 512 if c == 0 else 256
                        sl_ = slice(c * 512, c * 512 + w_)
                        pr = psb.tile([128, 512], f32, tag="psqk")
                        nc.tensor.matmul(pr[:, :w_], lhsT=pshuf, rhs=kb[:, sl_],
                                         start=True, stop=True)
                        t1 = rope_tmp.tile([128, 512], bf16, tag="ropet")
                        nc.vector.tensor_mul(out=t1[:, :w_], in0=pr[:, :w_],
                                             in1=msink[:, sl_])
                        nc.vector.tensor_add(out=kt_t[:, sl_], in0=u[:, sl_],
                                             in1=t1[:, :w_])

                qT.append(qt_t)
                kT.append(kt_t)
                return [c0, c1, c2, c3]

            def attn_front(mt, qt):
                """logits (PE) + exp (Act) + mask-mult (DVE) -> ET."""
                ps_l2 = []
                for hh in range(2):
                    hr = hh * 64
                    ps_l = psl.tile([128, 384], f32, tag="psl")
                    for j in range(3):
                        nc.tensor.matmul(
                            ps_l[:, j * 128:(j + 1) * 128],
                            lhsT=kT[mt][hr:hr + 64, (qt + j) * 128:(qt + j + 1) * 128],
                            rhs=qT[mt][hr:hr + 64, qt * 128:(qt + 1) * 128],
                            start=(j == 0), stop=(j == 2))
                    ps_l2.append(ps_l)
                Eb = at.tile([128, 2, 384], bf16, tag="Eb")
                for hh in range(2):
                    nc.scalar.activation(out=Eb[:, hh, :], in_=ps_l2[hh], func=AF.Exp,
                                         scale=float(HD) ** -0.5)
                ET = at.tile([128, 2, 384], bf16, tag="ET")
                for hh in range(2):
                    nc.vector.tensor_mul(out=ET[:, hh, :], in0=Eb[:, hh, :],
                                         in1=masks[qt])
                return (mt, qt, ET)

            def attn_back(ctx):
                """sums + AV (PE), then normalize into o28 (DVE)."""
                mt, qt, ET = ctx
                ps_s = pss.tile([128, 128], f32, tag="pss")
                ps_o = pso.tile([128, 128], f32, tag="pso")
                for hh in range(2):
                    hr = hh * 64
                    for j in range(3):
                        nc.tensor.matmul(ps_s[hr:hr + 64, :], lhsT=ones64,
                                         rhs=ET[:, hh, j * 128:(j + 1) * 128],
                                         start=(j == 0), stop=(j == 2))
                for hh in range(2):
                    hr = hh * 64
                    h = 2 * mt + hh
                    for j in range(3):
                        nc.tensor.matmul(ps_o[hr:hr + 64, :],
                                         lhsT=v_bf[qt + j][:, h * 64:h * 64 + 64],
                                         rhs=ET[:, hh, j * 128:(j + 1) * 128],
                                         start=(j == 0), stop=(j == 2))
                rec = at.tile([128, 128], f32, tag="rec")
                nc.vector.reciprocal(out=rec, in_=ps_s)
                nc.vector.tensor_mul(out=o28[mt // 2][:, mt % 2, qt * 128:(qt + 1) * 128],
                                     in0=ps_o, in1=rec)

            x2_list = [None] * NQT
            mv2_list = [None] * NQT
            y2_list = [None] * NQT

            def outproj_chunk(qt):
                def f():
                    x2 = x2_pool.tile([128, D], bf16, tag="x2")
                    for half in range(2):
                        ps = psb.tile([128, 512], f32, tag="psqk")
                        for cn in range(2):
                            reg = ps[:, cn * 256:(cn + 1) * 256]
                            for p in range(4):
                                nc.tensor.matmul(
                                    reg,
                                    lhsT=o28[p][:, :, qt * 128:(qt + 1) * 128],
                                    rhs=wo8[:, p, :,
                                            half * 512 + cn * 256:half * 512 + (cn + 1) * 256],
                                    start=(p == 0 and cn == 0), stop=(p == 3),
                                    perf_mode=DR)
                        sl = slice(half * 512, (half + 1) * 512)
                        nc.vector.tensor_add(out=x2[:, sl], in0=ps,
                                             in1=x_tiles[2 + qt][:, sl])
                    x2_list[qt] = x2
                    mv2_list[qt] = ln_stats(x2, ln_tmp2)
                return f

            def lnfin_chunk(qt):
                def f():
                    y2 = y2_pool.tile([128, D], bf16, tag="y2")
                    ln_norm(x2_list[qt], y2, mv2_list[qt], ln_tmp2)
                    y2_list[qt] = y2
                return f

            issue_wqk(0)
            issue_wqk(1)
            issue_wqk(2)
            chunks = proj_chunks(0)
            for c in chunks:
                c()
            fill_plan = {
                (7, 1): [outproj_chunk(0), lnfin_chunk(0)],
                (7, 2): [outproj_chunk(1), lnfin_chunk(1)],
                (7, 3): [outproj_chunk(2), lnfin_chunk(2)],
            }
            ctx = None
            for mt in range(8):
                if 3 <= mt + 3 < 8:
                    issue_wqk(mt + 3)
                if mt + 1 < 8:
                    nxt = proj_chunks(mt + 1)
                for qt in range(NQT):
                    nctx = attn_front(mt, qt)
                    if ctx is not None:
                        attn_back(ctx)
                    if mt + 1 < 8:
                        nxt[qt]()
                    else:
                        for fl in fill_plan.get((mt, qt), []):
                            fl()
                    ctx = nctx
            attn_back(ctx)
            outproj_chunk(NQT - 1)()
            lnfin_chunk(3)()

        qkv_scope.close()

        # ====== phases D: y2^T triplet interleaved with MLP gate/up ======
        H8 = []
        H8s = []
        with ExitStack() as ph:
            pst2 = ph.enter_context(tc.tile_pool(name="pst2", bufs=4, space="PSUM"))
            wgu_pool = ph.enter_context(tc.tile_pool(name="wgu", bufs=3))
            psg = ph.enter_context(tc.tile_pool(name="psg", bufs=4, space="PSUM"))
            gu_tmp = ph.enter_context(tc.tile_pool(name="gu_tmp", bufs=4))

            # y2^T split by query halves: a = tokens 0:256 (qt 0/1), b = 256:512
            y2T8a = [y2T_pool.tile([128, 2, 256], f8, name="y2T8a", tag="y2T8a")
                     for _ in range(4)]
            y2T8b = [y2T_pool.tile([128, 2, 256], f8, name="y2T8b", tag="y2T8b")
                     for _ in range(4)]
            y2T8sa = [y2T_pool.tile([128, 2, 256], f8, name="y2T8sa", tag="y2T8sa")
                      for _ in range(4)]
            y2T8sb = [y2T_pool.tile([128, 2, 256], f8, name="y2T8sb", tag="y2T8sb")
                      for _ in range(4)]
            y2Tr8a = [y2T_pool.tile([128, 2, 256], f8, name="y2Tr8a", tag="y2Tr8a")
                      for _ in range(4)] if GU_THIRD else None
            y2Tr8b = [y2T_pool.tile([128, 2, 256], f8, name="y2Tr8b", tag="y2Tr8b")
                      for _ in range(4)] if GU_THIRD else None
            gu_terms = [((y2T8a, y2T8b), 0), ((y2T8sa, y2T8sb), 1)] + \
                ([((y2Tr8a, y2Tr8b), 0)] if GU_THIRD else [])
            nterm = len(gu_terms)

            for pair in range(16):
                H8.append(hh_pool.tile([128, 2, CS], f8, name="H8", tag="hh"))
                H8s.append(hh_pool.tile([128, 2, CS], f8, name="H8s", tag="hhs"))

            def y2t_pass(half, hi_l, s_l, r_l):
                """Transpose qt pair (2*half, 2*half+1) for all 8 dtiles and
                ship the corresponding token-half fp8 triplet."""
                pts = {}
                for pair in range(4):
                    pts[pair] = pst2.tile([128, 2, 2, 128], bf16, name="pt2",
                                          tag="pst2b")
                for qi in range(2):
                    qt = half * 2 + qi
                    for pair in range(4):
                        for di in range(2):
                            dtl = pair * 2 + di
                            nc.tensor.transpose(
                                pts[pair][:, di, qi, :],
                                y2_list[qt][:, dtl * 128:(dtl + 1) * 128],
                                identb)
                for pair in range(4):
                    pt = pts[pair]
                    dst_hi = hi_l[pair][:, :, :]
                    nc.scalar.activation(out=dst_hi, in_=pt, func=AF.Identity)
                    nc.gpsimd.tensor_scalar_mul(out=s_l[pair][:, :, :],
                                                in0=dst_hi, scalar1=1.0 / 16)
                    if GU_THIRD:
                        nc.vector.tensor_tensor(out=r_l[pair][:, :, :],
                                                in0=pt, in1=dst_hi,
                                                op=OP.subtract)

            wgu_tiles = {}
            wd_tiles = {}

            def issue_wd(pair):
                w = wd_pool.tile([128, 2, 2, 1024], f8, name="wd", tag="wd")
                eng = nc.sync if pair % 2 == 0 else nc.gpsimd
                eng.dma_start(out=w, in_=wd_d[pair])
                wd_tiles[pair] = w

            def gu_mt_pass(mt, ps_pair, p, cns=(0, 1)):
                """K-pair accumulation pass of gate+up for f-block mt over the
                given column halves (cn 0 reads the a tiles, 1 the b)."""
                w = wgu_tiles[mt]
                for gi in range(2):
                    ps = ps_pair[gi]
                    for cn in cns:
                        reg = ps[:, cn * 256:(cn + 1) * 256]
                        for ti, (act, hl) in enumerate(gu_terms):
                            last = (p == 3 and ti == nterm - 1)
                            nc.tensor.matmul(
                                reg,
                                lhsT=w[:, gi, hl, p, :, :],
                                rhs=act[cn][p][:, :, :],
                                start=(p == 0 and ti == 0 and cn == 0),
                                stop=(last and not (has_bg and gi == 0)),
                                perf_mode=DR)

            def gu_mt_finish(mt, ps_pair):
                if has_bg:
                    for cn in range(2):
                        nc.tensor.matmul(
                            ps_pair[0][:, cn * 256:(cn + 1) * 256],
                            lhsT=bg_sb[:, mt * 128:(mt + 1) * 128],
                            rhs=ones_row[:, cn * 256:(cn + 1) * 256],
                            start=False, stop=True)
                U = gu_tmp.tile([128, CS], bf16, tag="U")
                nc.scalar.activation(out=U, in_=ps_pair[1], func=AF.Silu,
                                     bias=bu_sb[:, mt:mt + 1], scale=1.0)
                h8_dst = H8[mt // 2][:, mt % 2, :]
                nc.vector.tensor_mul(out=h8_dst, in0=ps_pair[0], in1=U)
                nc.gpsimd.tensor_scalar_mul(out=H8s[mt // 2][:, mt % 2, :],
                                            in0=h8_dst, scalar1=0.125)

            def new_gu_ps():
                return [psg.tile([128, CS], f32, name="psgu", tag="psgu")
                        for _ in range(2)]

            def issue_wgu(mt):
                pool = wgu0_pool if mt < 2 else wgu_pool
                w = pool.tile([128, 2, 2, 4, 2, 128], f8, name="wgu", tag="wgu")
                eng = nc.sync if mt % 2 == 0 else nc.gpsimd
                eng.dma_start(out=w, in_=wgu_d[mt])
                wgu_tiles[mt] = w

            # mt 0/1: pair passes interleaved with the y2T wave production so
            # PE stays fed while the transposes/copies stream out
            issue_wgu(0)
            issue_wgu(1)
            ps0, ps1 = new_gu_ps(), new_gu_ps()
            y2t_pass(0, y2T8a, y2T8sa, y2Tr8a)
            y2t_pass(1, y2T8b, y2T8sb, y2Tr8b)
            for p in range(4):
                gu_mt_pass(0, ps0, p, cns=(0,))
                gu_mt_pass(1, ps1, p, cns=(0,))
            for p in range(4):
                gu_mt_pass(0, ps0, p, cns=(1,))
                gu_mt_pass(1, ps1, p, cns=(1,))
            gu_mt_finish(0, ps0)
            gu_mt_finish(1, ps1)
            wgu_tiles.pop(0)
            wgu_tiles.pop(1)

            issue_wgu(2)
            for mt in range(2, 32):
                if mt + 1 < 32:
                    issue_wgu(mt + 1)
                if mt in (19, 21, 23, 25):
                    issue_wd((mt - 19) // 2)
                w = wgu_tiles[mt]
                psm = new_gu_ps()
                for p in range(4):
                    gu_mt_pass(mt, psm, p)
                gu_mt_finish(mt, psm)
                wgu_tiles.pop(mt)

        mid_scope.close()

        # ====== phase E: down proj (x4 weights) + residual + store ======
        with ExitStack() as ph:
            psd = ph.enter_context(tc.tile_pool(name="psd", bufs=8, space="PSUM"))
            out_pool = ph.enter_context(tc.tile_pool(name="outp", bufs=4))
            dn_tmp = ph.enter_context(tc.tile_pool(name="dn_tmp", bufs=4))

            ps_d = [psd.tile([128, 512], f32, name="psd", tag="psd") for _ in range(8)]
            dn_terms = [(H8, 0), (H8s, 1)]

            def dn_finish(tt):
                ot = out_pool.tile([128, D], f32, name="outp", tag="outp")
                for ch3 in range(2):
                    sl = slice(ch3 * 512, (ch3 + 1) * 512)
                    pd = ps_d[tt * 2 + ch3]
                    if has_bd:
                        # bias pre-scaled x4 on host to match the x4 weights
                        nc.tensor.matmul(pd[:, 0:256], lhsT=ones_row[:, 0:128],
                                         rhs=bd_sb[:, ch3 * 512:ch3 * 512 + 256],
                                         start=False, stop=True)
                        nc.tensor.matmul(pd[:, 256:512], lhsT=ones_row[:, 0:128],
                                         rhs=bd_sb[:, ch3 * 512 + 256:(ch3 + 1) * 512],
                                         start=False, stop=True)
                    tmp = dn_tmp.tile([128, 512], f32, name="dntmp", tag="dntmp")
                    nc.scalar.activation(out=tmp, in_=pd, func=AF.Identity,
                                         scale=0.25)
                    nc.vector.tensor_add(out=ot[:, sl], in0=tmp,
                                         in1=x2_list[tt][:, sl])
                eng2 = nc.sync if tt % 2 == 0 else nc.gpsimd
                eng2.dma_start(out=out_d[tt * 128:(tt + 1) * 128, :], in_=ot)

            for pair in range(16):
                if pair >= 1 and pair + 3 < 16:
                    issue_wd(pair + 3)
                w = wd_tiles.pop(pair)
                for tt in range(NQT):
                    for ti, (act, hl) in enumerate(dn_terms):
                        for cn in range(4):
                            reg = ps_d[tt * 2 + cn // 2][:, (cn % 2) * 256:
                                                         (cn % 2 + 1) * 256]
                            nc.tensor.matmul(
                                reg,
                                lhsT=act[pair][:, :, tt * 128:(tt + 1) * 128],
                                rhs=w[:, hl, :, cn * 256:(cn + 1) * 256],
                                start=(pair == 0 and ti == 0 and cn % 2 == 0),
                                stop=(pair == 15 and ti == 1 and not has_bd),
                                perf_mode=DR)
                    if pair == 15:
                        dn_finish(tt)
        mlp_scope.close()
        wqkp_scope.close()

    nc.compile()
    return nc


def prep_inputs(x, w_qkv, w_out, g1, b1, g2, b2, w_gate, b_gate, w_up, b_up,
                w_down, b_down):
    """Host-side: fold LN params, fp8-split weights, pre-tile, build per-core
    tensors."""
    import ml_dtypes
    f32 = np.float32
    bf16 = ml_dtypes.bfloat16
    f8 = ml_dtypes.float8_e4m3

    def split8(w, s=16.0):
        hi = w.astype(f8)
        lo = ((w - hi.astype(f32)) * s).astype(f8)
        return hi, lo

    wqkv_f = (w_qkv * g1[:, None]).astype(f32)
    bqkv = (b1 @ w_qkv).astype(f32)

    def qk_tile(w):  # [D, 1024] -> [mt, p, pair, i, m] fp8 pieces
        hi, lo = split8(w)
        t = lambda a: np.ascontiguousarray(
            a.reshape(4, 2, 128, 8, 128).transpose(3, 2, 0, 1, 4))
        return t(hi), t(lo)

    qhi, qlo = qk_tile(wqkv_f[:, :D])
    khi, klo = qk_tile(wqkv_f[:, D:2 * D])
    wqk = np.ascontiguousarray(
        np.stack([qhi, qlo, khi, klo], axis=2))  # [8,128,4,4,2,128]

    def mv_tile(w):  # [D, 1024] -> [p, pair, i, n]
        return w.reshape(4, 2, 128, 1024).transpose(2, 0, 1, 3)

    vhi, vlo = split8(wqkv_f[:, 2 * D:])
    wv = np.ascontiguousarray(np.stack([mv_tile(vhi), mv_tile(vlo)], axis=1))
    wo = np.ascontiguousarray(mv_tile(w_out.astype(f32).astype(f8)))

    def gu_tile(w):  # [D, F] -> [mt, p, hi/lo, pair, i, m]
        hi, lo = split8(w)
        t = lambda a: a.reshape(4, 2, 128, 32, 128).transpose(3, 2, 0, 1, 4)
        return np.stack([t(hi), t(lo)], axis=2)  # [32,128,2,4,2,128]

    wg_f = (w_gate * g2[:, None]).astype(f32)
    wu_f = (w_up * g2[:, None]).astype(f32)
    wgu = np.ascontiguousarray(
        np.stack([gu_tile(wg_f), gu_tile(wu_f)], axis=2))  # [32,128,2,2,4,2,128]

    wd_f = w_down.astype(f32)
    wd_hi = (4.0 * wd_f).astype(f8)
    wd_lo = (32.0 * (wd_f - wd_hi.astype(f32) / 4.0)).astype(f8)
    t_wd = lambda a: a.reshape(16, 2, 128, 1024).transpose(0, 2, 1, 3)
    wd = np.ascontiguousarray(np.stack([t_wd(wd_hi), t_wd(wd_lo)], axis=2))

    bqk_pt = bqkv[:2048].reshape(16, 128).T                       # [p, t]
    bu_pt = (b_up + b2 @ w_up).astype(f32).reshape(32, 128).T
    cbf = np.ascontiguousarray(
        np.concatenate([bqk_pt, bu_pt], axis=1)).astype(f32)      # [128, 48]

    bg_row = (b_gate + b2 @ w_gate).astype(f32).reshape(1, F).astype(bf16)
    bv_row = bqkv[2048:].reshape(1, D).astype(bf16)
    bd_row = (4.0 * b_down).reshape(1, D).astype(bf16)

    # rotate-half permutation (sign folded into sin tables)
    pshuf = np.zeros((128, 128), f32)
    for m in range(128):
        base = (m // 64) * 64
        r = m % 64
        sig = base + (r + 32) % 64
        pshuf[sig, m] = 1.0
    pshuf = pshuf.astype(bf16)

    half = HD // 2
    inv_freq = 1.0 / (10000.0 ** (np.arange(half, dtype=np.float64) / half))

    def rope_tables(pos):
        t = np.maximum(pos, 0).astype(np.float64)
        freqs = np.outer(t, inv_freq)
        emb = np.concatenate([freqs, freqs], 1)
        c = np.cos(emb).T.astype(f32)
        s = np.sin(emb).T.astype(f32)
        ms = s.copy()
        ms[:32] = -ms[:32]
        return (np.ascontiguousarray(np.vstack([c, c])),
                np.ascontiguousarray(np.vstack([ms, ms])))

    common = {"wqk": wqk, "wv": wv, "wo": wo, "wgu": wgu, "wd": wd,
              "bv": bv_row, "bd": bd_row, "bg": bg_row, "cbf": cbf}

    in_maps = []
    for c in range(NCORES):
        b, chunk = c // CH, c % CH
        q0 = chunk * CS
        lo = q0 - WIN
        xh = np.zeros((HT, D), f32)
        src_lo = max(0, lo)
        xh[src_lo - lo:] = x[b, src_lo:q0 + CS]
        xh = xh.astype(bf16)
        pos_k = np.arange(lo, q0 + CS)
        cosk_a, sink_a = rope_tables(pos_k)
        cosq_a = np.ascontiguousarray(cosk_a[:, WIN:]).astype(bf16)
        sinq_a = np.ascontiguousarray(sink_a[:, WIN:]).astype(bf16)
        # transposed multiplicative mask [r, qt, j, c]:
        #   key j_g = lo + (qt+j)*128 + r ; query i = q0 + qt*128 + c
        r_i = np.arange(128)[:, None, None, None]
        qt_i = np.arange(NQT)[None, :, None, None]
        j_i = np.arange(3)[None, None, :, None]
        c_i = np.arange(128)[None, None, None, :]
        jg = lo + (qt_i + j_i) * 128 + r_i
        gi = q0 + qt_i * 128 + c_i
        valid = (jg <= gi) & (gi - jg <= WIN) & (jg >= 0)
        maskT = valid.astype(f32).reshape(128, NQT * 3 * 128).astype(bf16)
        cbb = np.concatenate(
            [cosq_a, sinq_a, cosk_a.astype(bf16), sink_a.astype(bf16),
             maskT, pshuf], axis=1)
        in_maps.append(dict(common, xh=xh, cbb=np.ascontiguousarray(cbb)))
    return in_maps


_PROG = {}


def kernel(**inputs):
    from concourse.bass_utils import run_bass_kernel_spmd

    inputs = {k: np.asarray(v, dtype=np.float32) for k, v in inputs.items()}
    in_maps = prep_inputs(**inputs)
    flags = (bool(np.any(inputs["b1"] @ inputs["w_qkv"][:, 2048:])),
             bool(np.any(inputs["b_gate"] + inputs["b2"] @ inputs["w_gate"])),
             bool(np.any(inputs["b_down"])))
    if flags not in _PROG:
        _PROG[flags] = build_program(has_bv=flags[0], has_bg=flags[1],
                                     has_bd=flags[2])
    nc = _PROG[flags]
    res = run_bass_kernel_spmd(nc, in_maps, core_ids=list(range(NCORES)))
    out = np.zeros((B, S, D), np.float32)
    for c in range(NCORES):
        b, chunk = c // CH, c % CH
        out[b, chunk * CS:(chunk + 1) * CS] = res.results[c]["out"]
    return out
